# revision 26
# baseline (speedup 1.0000x reference)
"""4-layer GATv2 forward pass on 8 TRN2 NeuronCores (Bass/Tile).

Strategy (node/dst partitioning, no cross-core segment reductions):
  - Nodes are padded to 20480 and split into 8 contiguous slices of 2560
    (20 blocks of 128 dst nodes per core).  Each core owns the segment
    softmax + weighted scatter for its dst nodes, so all softmax
    reductions are core-local.
  - Edges (with self loops appended) are routed to the core/block that
    owns their dst.  Per (core, block) edge counts are padded to a
    shared multiple of 128 (G[b] groups of 128 edges) so one NEFF works
    for all 8 cores.
  - Layer 1 source transforms (xl) are computed for the FULL node table
    on every core (x is replicated), so no collective is needed before
    the first edge phase.  For layers 2-4, the matmul for layer l+1 of
    dst block b is interleaved right after the edge phase of block b
    (its input h lives in SBUF), so the AllGather of the next xl table
    can issue the moment the edge phase drains.
  - Per-edge work is edge-major (partition = edge % 128): dma_gather of
    xl[src] rows (parallel calls over the 4 SWDGE queues), xr[dst]
    expansion + z = xl+xr + leaky-relu via PE matmuls against a one-hot
    dst selector, per-head dot with `a` (DVE folds+reduce), exp, then
    the softmax denominator and alpha-weighted sum of xl[src] as one PE
    matmul per 128-edge group (SelT).
  - Softmax uses exp(logit) directly (no running max): logits are O(10)
    here, fp32 exp is exact enough, and the math is identical to the
    reference's shifted softmax.

kernel(**inputs) takes the full problem inputs and returns the full
[20000, 16] fp32 output.
"""

import numpy as np

import concourse.bass as bass
import concourse.bacc as bacc
import concourse.mybir as mybir
import concourse.tile as tile
from concourse.bass_utils import run_bass_kernel_spmd
from concourse.masks import make_identity

F16 = mybir.dt.float16
BF16 = mybir.dt.bfloat16
F32 = mybir.dt.float32
I16 = mybir.dt.int16
U8 = mybir.dt.uint8
P = 128

# model dims (fixed by the problem)
N_REAL = 20000
E_RAW = 320000
IN_CH = 128
HID = 64
HEADS = 4
OUT_CH = 16
SLOPE = 0.2

MASK_NEG = -50.0  # additive logit bias for pad edges
DEN_EPS = 1e-12   # keeps reciprocal() in range for edgeless (pad) dst rows


class Cfg:
    def __init__(self, n_cores, npc, n_real, layers, out_real):
        assert npc % P == 0
        self.n_cores = n_cores
        self.npc = npc              # nodes per core (padded)
        self.nblk = npc // P        # dst blocks per core
        self.n_real = n_real
        self.npad = n_cores * npc
        self.layers = layers        # list of dicts: c_in, c_tbl, n_h, c_h
        self.out_real = out_real    # real output channels of last layer


def real_cfg():
    layers = [
        dict(c_in=IN_CH, c_tbl=HEADS * HID, c_cmp=HEADS * HID,
             n_h=HEADS, c_h=HID),
        dict(c_in=HEADS * HID, c_tbl=HEADS * HID, c_cmp=HEADS * HID,
             n_h=HEADS, c_h=HID),
        dict(c_in=HEADS * HID, c_tbl=HEADS * HID, c_cmp=HEADS * HID,
             n_h=HEADS, c_h=HID),
        # 16 real out channels: gather table stays 128 wide (256B descriptor
        # minimum) but all edge-phase compute runs at width 32
        dict(c_in=HEADS * HID, c_tbl=P, c_cmp=32, n_h=1, c_h=32),
    ]
    return Cfg(8, 2560, N_REAL, layers, OUT_CH)


# ---------------------------------------------------------------------------
# host-side graph preprocessing
# ---------------------------------------------------------------------------

def prep_graph(cfg, edge_index):
    """Route edges (plus self loops) to (core, block) by dst; build per-core
    gather-index / dst-local / mask arrays in the exact SBUF layouts the
    kernel consumes.

    Dst nodes are REASSIGNED to (core, block) bins by LPT load balancing
    (highest in-degree first, always into the lightest non-full bin) so
    per-bin edge counts are near-uniform: the shared padded group count
    drops from ceil(max/128) to ceil(mean/128).  All device-side arrays
    are expressed in permuted "position" space; kernel() un-permutes the
    output rows at the end (meta["ipos"])."""
    n = cfg.n_real
    src = np.concatenate([np.asarray(edge_index[0], np.int64),
                          np.arange(n, dtype=np.int64)])
    dst = np.concatenate([np.asarray(edge_index[1], np.int64),
                          np.arange(n, dtype=np.int64)])
    assert src.min() >= 0 and src.max() < n and dst.min() >= 0 and dst.max() < n

    nbin = cfg.n_cores * cfg.nblk
    deg = np.bincount(dst, minlength=n)
    nodes_by_deg = np.argsort(-deg, kind="stable")
    binsum = np.zeros(nbin, np.int64)
    bincnt = np.zeros(nbin, np.int64)
    ipos = np.empty(n, np.int64)
    perm = np.full(cfg.npad, -1, np.int64)
    for v in nodes_by_deg:
        open_b = np.flatnonzero(bincnt < P)
        b = open_b[np.argmin(binsum[open_b])]
        p_ = b * P + bincnt[b]
        ipos[v] = p_
        perm[p_] = v
        binsum[b] += deg[v]
        bincnt[b] += 1
    meta = dict(perm=perm, ipos=ipos)

    src, dst = ipos[src], ipos[dst]       # positions from here on
    gblk = dst // P                       # global block id (core-major)
    order = np.argsort(gblk, kind="stable")
    src, dst, gblk = src[order], dst[order], gblk[order]

    nblk_tot = cfg.n_cores * cfg.nblk
    counts = np.bincount(gblk, minlength=nblk_tot).reshape(cfg.n_cores, cfg.nblk)
    G = np.maximum(1, (counts.max(axis=0) + P - 1) // P).astype(np.int64)  # [nblk]
    W = int(G.sum())

    # split edges per (core, block)
    starts = np.zeros(nblk_tot + 1, np.int64)
    np.cumsum(counts.reshape(-1), out=starts[1:])

    per_core = []
    for c in range(cfg.n_cores):
        xl_idx = np.zeros((P, 8 * W), np.int16)
        dloc = np.zeros((P, W), np.float16)
        dlocT = np.zeros((1, W * P), np.float16)
        mbias = np.full((P, W), MASK_NEG, np.float16)
        off = 0
        for b in range(cfg.nblk):
            gb = c * cfg.nblk + b
            s, e = starts[gb], starts[gb + 1]
            nreal = int(e - s)
            npad_e = int(G[b]) * P
            fsrc = np.zeros(npad_e, np.int64)
            fdl = np.zeros(npad_e, np.int64)
            fm = np.full(npad_e, MASK_NEG, np.float32)
            fsrc[:nreal] = src[s:e]
            fdl[:nreal] = dst[s:e] % P
            fm[:nreal] = 0.0
            # edge i -> partition i % 128, group i // 128
            dloc[:, off:off + G[b]] = fdl.reshape(G[b], P).T.astype(np.float16)
            dlocT[0, off * P:(off + int(G[b])) * P] = fdl.astype(np.float16)
            mbias[:, off:off + G[b]] = fm.reshape(G[b], P).T.astype(np.float16)
            # wrapped idx layout: wrapped[p, s] = flat[s*16 + p], replicated
            # into all 8 16-partition groups (one per GPSIMD Q7 core)
            xl_idx[:, 8 * off:8 * (off + G[b])] = np.tile(
                fsrc.astype(np.int16).reshape(-1, 16).T, (8, 1))
            off += int(G[b])
        per_core.append(dict(xl_idx=xl_idx, dloc=dloc,
                             dlocT=np.tile(dlocT, (P, 1)), mbias=mbias))
    return [int(g) for g in G], per_core, meta


# ---------------------------------------------------------------------------
# bass program
# ---------------------------------------------------------------------------

def build_nc(cfg, G):
    """Build the (single, SPMD) bass program."""
    nl = len(cfg.layers)
    W = sum(G)
    Gmax = max(G)
    c_tbl_max = max(L["c_tbl"] for L in cfg.layers)
    kc_max = max(L["c_in"] for L in cfg.layers) // P
    nblk_tbl = cfg.npad // P            # full-table blocks (layer-1 xl)

    nc = bacc.Bacc("TRN2", target_bir_lowering=False, debug=False,
                   num_devices=cfg.n_cores, num_swdge_queues=4)

    # layer-1 input, pre-transposed on the host: [c_in, nodes]
    x_fullT = nc.dram_tensor("x_fullT", [cfg.layers[0]["c_in"], cfg.npad], F16,
                             kind="ExternalInput")
    x_ownT = nc.dram_tensor("x_ownT", [cfg.layers[0]["c_in"], cfg.npc], F16,
                            kind="ExternalInput")
    xl_idx_d = nc.dram_tensor("xl_idx", [P, 8 * W], I16, kind="ExternalInput")
    dloc_d = nc.dram_tensor("dloc", [P, W], F16, kind="ExternalInput")
    dlocT_d = nc.dram_tensor("dlocT", [P, W * P], F16, kind="ExternalInput")
    iotac_d = nc.dram_tensor("iotac", [P, Gmax * P], F16, kind="ExternalInput")
    mbias_d = nc.dram_tensor("mbias", [P, W], F16, kind="ExternalInput")
    iota_d = nc.dram_tensor("iota", [P, P], F16, kind="ExternalInput")
    w_d, a_d = [], []
    for l, L in enumerate(cfg.layers):
        wl = nc.dram_tensor(f"w{l}l", [L["c_in"], L["c_tbl"]], F16,
                            kind="ExternalInput")
        wr = nc.dram_tensor(f"w{l}r", [L["c_in"], L["c_cmp"]], F16,
                            kind="ExternalInput")
        w_d.append((wl, wr))
        a_d.append(nc.dram_tensor(f"a{l}", [P, L["c_cmp"]], F16,
                                  kind="ExternalInput"))
    out_d = nc.dram_tensor("out", [cfg.npc, cfg.out_real], F32,
                           kind="ExternalOutput")
    import os
    dbg = os.environ.get("K_DEBUG") == "1"
    if dbg:
        dbg_xl0 = nc.dram_tensor("dbg_xl0", [cfg.npc, cfg.layers[0]["c_tbl"]],
                                 F16, kind="ExternalOutput")
        dbg_xl1 = nc.dram_tensor("dbg_xl1", [cfg.npc, cfg.layers[1]["c_tbl"]],
                                 F16, kind="ExternalOutput")
        dbg_xr1 = nc.dram_tensor("dbg_xr1", [P, cfg.nblk * 256], F16,
                                 kind="ExternalOutput")
        dbg_h1 = nc.dram_tensor("dbg_h1", [cfg.npc, 256], F16,
                                 kind="ExternalOutput")
        dbg_lrz = nc.dram_tensor("dbg_lrz", [P, 18 * 256], F16,
                                 kind="ExternalOutput")
        dbg_xlg = nc.dram_tensor("dbg_xlg", [P, 18 * 256], F16,
                                 kind="ExternalOutput")
        dbg_ex = nc.dram_tensor("dbg_ex", [P, 18 * HEADS], F16,
                                kind="ExternalOutput")

    rg = [list(range(cfg.n_cores))]

    with tile.TileContext(nc) as tc:
        with (
            tc.tile_pool(name="const", bufs=1) as cpool,
            tc.tile_pool(name="wts", bufs=2) as wpool,
            tc.tile_pool(name="mm", bufs=3) as mpool,
            tc.tile_pool(name="gath", bufs=3) as gpool,
            tc.tile_pool(name="gidx", bufs=12) as gipool,
            tc.tile_pool(name="edge", bufs=2) as epool,
            tc.tile_pool(name="small", bufs=2) as spool,
            tc.tile_pool(name="hbuf", bufs=3) as hpool,
            tc.tile_pool(name="xrsb", bufs=2) as xpool,
            tc.tile_pool(name="psum", bufs=2, space="PSUM") as ppool,
            tc.tile_pool(name="dram", bufs=1, space="DRAM") as dpool,
        ):
            # ---- persistent constants -------------------------------------
            iota_sb = cpool.tile([P, P], F16, tag="iota")
            nc.sync.dma_start(out=iota_sb[:], in_=iota_d[:])
            ident = cpool.tile([P, P], F16, tag="ident")
            make_identity(nc, ident[:])
            dloc_sb = cpool.tile([P, W], F16, tag="dloc")
            nc.sync.dma_start(out=dloc_sb[:], in_=dloc_d[:])
            mb_sb = cpool.tile([P, W], F16, tag="mbias")
            nc.sync.dma_start(out=mb_sb[:], in_=mbias_d[:])
            # iotac_u8[p, e] = p  (host-replicated)
            iotac_sb = cpool.tile([P, Gmax * P], F16, tag="iotacr")
            nc.sync.dma_start(out=iotac_sb[:], in_=iotac_d[:])
            if dbg:
                dbg_ex_sb = cpool.tile([P, 18 * HEADS], F16, tag="dbgex")

            # ---- per-layer DRAM scratch -----------------------------------
            xl_loc, xl_tbl = [None], [None]
            # layer 0 xl table is computed fully on every core -> Local.
            xl_tbl0 = dpool.tile([cfg.npad, cfg.layers[0]["c_tbl"]], F16,
                                 tag="xltbl0", name="xltbl0")
            for l in range(1, nl):
                L = cfg.layers[l]
                xl_loc.append(dpool.tile([cfg.npc, L["c_tbl"]], F16,
                                         tag=f"xlloc{l}", name=f"xlloc{l}"))
                xl_tbl.append(dpool.tile(
                    [cfg.npad, L["c_tbl"]], F16, tag=f"xltbl{l}",
                    name=f"xltbl{l}", addr_space="Shared"))
            xl_tbl[0] = xl_tbl0

            # per-layer xr tables stay in SBUF (written by the interleaved
            # matmul of the previous layer's edge phase)
            xr_sb = [xpool.tile([P, cfg.nblk * c_tbl_max], F16, tag="xr_sb",
                                name=f"xr_sb{l}")
                     for l in range(nl)]

            # weight tiles per layer (wpool rotates 2 buffers)
            def load_weights(l):
                L = cfg.layers[l]
                Cl, Cr, c_in = L["c_tbl"], L["c_cmp"], L["c_in"]
                kc_n = c_in // P
                wl_sb = wpool.tile([P, kc_max * c_tbl_max], F16, tag="wl")
                wr_sb = wpool.tile([P, kc_max * c_tbl_max], F16, tag="wr")
                for kc in range(kc_n):
                    nc.sync.dma_start(out=wl_sb[:, kc * Cl:(kc + 1) * Cl],
                                      in_=w_d[l][0][kc * P:(kc + 1) * P, :])
                    nc.sync.dma_start(out=wr_sb[:, kc * Cr:(kc + 1) * Cr],
                                      in_=w_d[l][1][kc * P:(kc + 1) * P, :])
                return wl_sb, wr_sb

            def load_arep(l):
                L = cfg.layers[l]
                C = L["c_cmp"]
                a_rep = wpool.tile([P, Gmax * c_tbl_max], F16, tag="arep")
                nc.sync.dma_start(
                    out=a_rep[:, :Gmax * C].rearrange("p (g c) -> p g c", g=Gmax),
                    in_=a_d[l][:].rearrange("p (g c) -> p g c", g=1)
                        .to_broadcast([P, Gmax, C]))
                return a_rep

            def mm_block(l, wl_sb, wr_sb, h_ap, blk, do_xl, do_xr):
                """Source/target transforms of one 128-node block of layer l.
                h_ap: [P, c_in] SBUF activation tile; writes xl to
                xl_loc/xl_tbl and/or xr into the resident xr_sb table."""
                L = cfg.layers[l]
                Cl, Cr, c_in = L["c_tbl"], L["c_cmp"], L["c_in"]
                kc_n = c_in // P
                hT = mpool.tile([P, kc_max * P], F16, tag="hT")
                for kc in range(kc_n):
                    pt = ppool.tile([P, P], F16, tag="pt", bufs=1)
                    nc.tensor.transpose(pt[:], h_ap[:, kc * P:(kc + 1) * P],
                                        ident[:])
                    nc.vector.tensor_copy(out=hT[:, kc * P:(kc + 1) * P],
                                          in_=pt[:])
                if do_xl:
                    ps_xl = ppool.tile([P, c_tbl_max], F32, tag="ps_mm")
                    for kc in range(kc_n):
                        nc.tensor.matmul(ps_xl[:, :Cl],
                                         lhsT=hT[:, kc * P:(kc + 1) * P],
                                         rhs=wl_sb[:, kc * Cl:(kc + 1) * Cl],
                                         start=(kc == 0), stop=(kc == kc_n - 1))
                    xl_t = mpool.tile([P, c_tbl_max], F16, tag="xl_t")
                    nc.scalar.activation(xl_t[:, :Cl], ps_xl[:, :Cl],
                                         mybir.ActivationFunctionType.Copy)
                    if l == 0:
                        nc.sync.dma_start(
                            out=xl_tbl0[blk * P:(blk + 1) * P, :],
                            in_=xl_t[:, :Cl])
                    else:
                        nc.sync.dma_start(
                            out=xl_loc[l][blk * P:(blk + 1) * P, :],
                            in_=xl_t[:, :Cl])
                if do_xr:
                    ps_xr = ppool.tile([P, c_tbl_max], F32, tag="ps_mm")
                    for kc in range(kc_n):
                        nc.tensor.matmul(ps_xr[:, :Cr],
                                         lhsT=hT[:, kc * P:(kc + 1) * P],
                                         rhs=wr_sb[:, kc * Cr:(kc + 1) * Cr],
                                         start=(kc == 0), stop=(kc == kc_n - 1))
                    nc.scalar.activation(
                        xr_sb[l][:, blk * c_tbl_max:blk * c_tbl_max + Cr],
                        ps_xr[:, :Cr], mybir.ActivationFunctionType.Copy)

            # ---- layer 0 matmul phase: full xl table, local xr ------------
            # x arrives pre-transposed, so each block's lhsT is a direct
            # slice load: no PE transposes, batched 4-block DMAs.
            wl_sb, wr_sb = load_weights(0)
            c_in0 = cfg.layers[0]["c_in"]
            C0 = cfg.layers[0]["c_tbl"]
            TB = 8
            for t0 in range(0, nblk_tbl, TB):
                xT4 = mpool.tile([P, TB * P], F16, tag="xT4")
                nc.sync.dma_start(out=xT4[:],
                                  in_=x_fullT[:, t0 * P:(t0 + TB) * P])
                xl_t4 = mpool.tile([P, TB * C0], F16, tag="xl_t4")
                for ti in range(TB):
                    ps_xl = ppool.tile([P, c_tbl_max], F32, tag="ps_mm")
                    nc.tensor.matmul(ps_xl[:, :C0],
                                     lhsT=xT4[:, ti * P:(ti + 1) * P],
                                     rhs=wl_sb[:, :C0], start=True, stop=True)
                    if ti % 2 == 0:
                        nc.scalar.activation(xl_t4[:, ti * C0:(ti + 1) * C0],
                                             ps_xl[:, :C0],
                                             mybir.ActivationFunctionType.Copy)
                    else:
                        nc.vector.tensor_copy(
                            out=xl_t4[:, ti * C0:(ti + 1) * C0],
                            in_=ps_xl[:, :C0])
                nc.sync.dma_start(
                    out=xl_tbl0[t0 * P:(t0 + TB) * P, :]
                        .rearrange("(t p) c -> p t c", p=P),
                    in_=xl_t4[:].rearrange("p (t c) -> p t c", t=TB))
            TBo = 4
            for b0 in range(0, cfg.nblk, TBo):
                xT4 = mpool.tile([P, TB * P], F16, tag="xT4")
                nc.sync.dma_start(out=xT4[:, :TBo * P],
                                  in_=x_ownT[:, b0 * P:(b0 + TBo) * P])
                for bi in range(TBo):
                    b = b0 + bi
                    ps_xr = ppool.tile([P, c_tbl_max], F32, tag="ps_mm")
                    nc.tensor.matmul(ps_xr[:, :C0],
                                     lhsT=xT4[:, bi * P:(bi + 1) * P],
                                     rhs=wr_sb[:, :C0], start=True, stop=True)
                    nc.scalar.activation(
                        xr_sb[0][:, b * c_tbl_max:b * c_tbl_max + C0],
                        ps_xr[:, :C0], mybir.ActivationFunctionType.Copy)

            # ---- per-layer edge phase (+ interleaved next-layer matmul) ---
            qn = [0]

            def gather_rows(tbl_ap, out_tile, off_g, n_g, C_, nm):
                """Gather n_g*128 rows from tbl_ap into out_tile
                [P, n_g, C_], split into <=6-group chunks spread over the
                4 SWDGE queues (no ordering chain: they run concurrently)."""
                CH = 6
                for k0 in range(0, n_g, CH):
                    gk = min(CH, n_g - k0)
                    it = gipool.tile([P, 8 * CH], I16, tag="idxt",
                                     name=f"idxt_{nm}_{k0}")
                    nc.sync.dma_start(
                        out=it[:, :8 * gk],
                        in_=xl_idx_d[:, 8 * (off_g + k0):
                                     8 * (off_g + k0 + gk)])
                    nc.gpsimd.dma_gather(
                        out_ap=out_tile[:, k0 * C_:(k0 + gk) * C_]
                            .rearrange("p (g c) -> p g c", c=C_),
                        in_ap=tbl_ap,
                        idxs_ap=it[:, :8 * gk],
                        num_idxs=gk * P, num_idxs_reg=gk * P,
                        elem_size=C_, queue_num=qn[0] % 4)
                    qn[0] += 1

            for l, L in enumerate(cfg.layers):
                c_in, C, n_h, c_h = L["c_in"], L["c_cmp"], L["n_h"], L["c_h"]
                CT = L["c_tbl"]
                EC = C + n_h
                a_rep = load_arep(l)
                if l + 1 < nl:
                    wl_nxt, wr_nxt = load_weights(l + 1)

                for b in range(cfg.nblk):
                    gG = G[b]
                    off = sum(G[:b])
                    xl_g = gpool.tile([P, Gmax * c_tbl_max], F16, tag="xl_g")
                    gather_rows(xl_tbl[l][:, :], xl_g, off, gG, CT,
                                f"xl{l}_{b}")
                    xl3 = xl_g[:, :gG * CT].rearrange(
                        "p (g c) -> p g c", c=CT)
                    xr_blk = xr_sb[l][:, b * c_tbl_max:b * c_tbl_max + C]
                    # Sel[d, e] = (d == dloc[e])    [for xr expansion]
                    dlt = epool.tile([P, Gmax * P], F16, tag="dlt")
                    nc.sync.dma_start(out=dlt[:, :gG * P],
                                      in_=dlocT_d[:, off * P:(off + gG) * P])
                    sel = epool.tile([P, Gmax * P], F16, tag="sel")
                    nc.vector.tensor_tensor(
                        out=sel[:, :gG * P], in0=dlt[:, :gG * P],
                        in1=iotac_sb[:, :gG * P],
                        op=mybir.AluOpType.is_equal)
                    # SelT[e, d] = (dloc[e] == d)   [for num/den matmuls]
                    selt = epool.tile([P, Gmax * P], BF16, tag="selt")
                    nc.vector.tensor_tensor(
                        out=selt[:, :gG * P].rearrange(
                            "p (g d) -> p g d", d=P),
                        in0=dloc_sb[:, off:off + gG]
                            .rearrange("p (g d) -> p g d", d=1)
                            .to_broadcast([P, gG, P]),
                        in1=iota_sb[:].rearrange("p (g d) -> p g d", g=1)
                            .to_broadcast([P, gG, P]),
                        op=mybir.AluOpType.is_equal)
                    # z (per quad of groups) in PSUM:
                    #   z_g = Sel_g^T @ xr_blk + I^T @ xl_g   -> leaky relu
                    lrz = epool.tile([P, Gmax * c_tbl_max], F16, tag="lrz")
                    for g0 in range(0, gG, 2):
                        gns = min(2, gG - g0)
                        ps_z = ppool.tile([P, 2 * c_tbl_max], F32, tag="ps_z",
                                          bufs=3)
                        # NOTE: each slice's start->stop matmul pair must stay
                        # tightly sequential; interleaving several open
                        # accumulation groups corrupts PSUM on this HW.
                        for gg in range(g0, g0 + gns):
                            sl = slice((gg - g0) * C, (gg - g0 + 1) * C)
                            nc.tensor.matmul(
                                ps_z[:, sl], lhsT=sel[:, gg * P:(gg + 1) * P],
                                rhs=xr_blk, start=True, stop=False)
                            nc.tensor.matmul(
                                ps_z[:, sl], lhsT=ident[:],
                                rhs=xl_g[:, gg * CT:gg * CT + C],
                                start=False, stop=True)
                        nc.scalar.activation(
                            lrz[:, g0 * C:(g0 + gns) * C],
                            ps_z[:, :gns * C],
                            mybir.ActivationFunctionType.Prelu,
                            alpha=SLOPE)
                    # a * LR(z)
                    alr = epool.tile([P, Gmax * c_tbl_max], F16, tag="alr")
                    nc.vector.tensor_tensor(out=alr[:, :gG * C],
                                            in0=lrz[:, :gG * C],
                                            in1=a_rep[:, :gG * C],
                                            op=mybir.AluOpType.mult)
                    # logits: two folds + reduce over c_h/4
                    ch2, ch4 = c_h // 2, c_h // 4
                    fold1 = spool.tile([P, Gmax * c_tbl_max // 2], F16,
                                       tag="fold1")
                    a4 = alr[:, :gG * C].rearrange(
                        "p (g h c) -> p g h c", h=n_h, c=c_h)
                    f13 = fold1[:, :gG * C // 2].rearrange(
                        "p (g h c) -> p g h c", h=n_h, c=ch2)
                    nc.vector.tensor_tensor(out=f13, in0=a4[:, :, :, :ch2],
                                            in1=a4[:, :, :, ch2:],
                                            op=mybir.AluOpType.add)
                    fold2 = spool.tile([P, Gmax * c_tbl_max // 4], F16,
                                       tag="fold2")
                    f23 = fold2[:, :gG * C // 4].rearrange(
                        "p (g h c) -> p g h c", h=n_h, c=ch4)
                    nc.vector.tensor_tensor(out=f23, in0=f13[:, :, :, :ch4],
                                            in1=f13[:, :, :, ch4:],
                                            op=mybir.AluOpType.add)
                    logits = spool.tile([P, Gmax * HEADS], F32, tag="logits")
                    nc.vector.tensor_reduce(
                        out=logits[:, :gG * n_h].rearrange(
                            "p (g h) -> p g h", h=n_h),
                        in_=f23,
                        axis=mybir.AxisListType.X, op=mybir.AluOpType.add)
                    # pad-edge mask as additive bias
                    logm = spool.tile([P, Gmax * HEADS], F32, tag="logm")
                    nc.vector.tensor_tensor(
                        out=logm[:, :gG * n_h].rearrange(
                            "p (g h) -> p g h", h=n_h),
                        in0=logits[:, :gG * n_h].rearrange(
                            "p (g h) -> p g h", h=n_h),
                        in1=mb_sb[:, off:off + gG]
                            .rearrange("p (g h) -> p g h", h=1)
                            .to_broadcast([P, gG, n_h]),
                        op=mybir.AluOpType.add)
                    ex = spool.tile([P, Gmax * HEADS], BF16, tag="ex")
                    nc.scalar.activation(ex[:, :gG * n_h], logm[:, :gG * n_h],
                                         mybir.ActivationFunctionType.Exp)
                    # expand ex over c_h on the scalar engine (it has
                    # headroom; the packed multiply then runs at DVE 2x)
                    ex_e = epool.tile([P, Gmax * c_tbl_max], BF16, tag="ex_e")
                    nc.scalar.activation(
                        ex_e[:, :gG * C].rearrange(
                            "p (g h c) -> p g h c", h=n_h, c=c_h),
                        ex[:, :gG * n_h].rearrange(
                            "p (g h c) -> p g h c", h=n_h, c=1)
                            .to_broadcast([P, gG, n_h, c_h]),
                        mybir.ActivationFunctionType.Copy)
                    # edata = [ex * xl[src] | ex]
                    edata = epool.tile([P, Gmax * (c_tbl_max + HEADS)], BF16,
                                       tag="edata")
                    ed3 = edata[:, :gG * EC].rearrange("p (g c) -> p g c", c=EC)
                    nc.vector.tensor_tensor(
                        out=ed3[:, :, :C],
                        in0=xl3[:, :, :C],
                        in1=ex_e[:, :gG * C].rearrange("p (g c) -> p g c", c=C),
                        op=mybir.AluOpType.mult)
                    nc.vector.tensor_copy(
                        out=ed3[:, :, C:],
                        in_=ex[:, :gG * n_h].rearrange("p (g h) -> p g h",
                                                       h=n_h))
                    # segment sums via PE: psum[d, :] += SelT_g^T @ edata_g
                    ps_nd = ppool.tile([P, EC], F32, tag="ps_nd", bufs=2)
                    for g in range(gG):
                        nc.tensor.matmul(
                            ps_nd[:], lhsT=selt[:, g * P:(g + 1) * P],
                            rhs=edata[:, g * EC:(g + 1) * EC],
                            start=(g == 0), stop=(g == gG - 1))
                    den_e = spool.tile([P, HEADS], F32, tag="den_e")
                    nc.vector.tensor_scalar(
                        out=den_e[:, :n_h], in0=ps_nd[:, C:], scalar1=DEN_EPS,
                        scalar2=None, op0=mybir.AluOpType.add)
                    rden = spool.tile([P, HEADS], F32, tag="rden")
                    nc.vector.reciprocal(rden[:, :n_h], den_e[:, :n_h])
                    ob = spool.tile([P, c_tbl_max], F32, tag="ob")
                    nc.vector.tensor_tensor(
                        out=ob[:, :C].rearrange("p (h c) -> p h c", h=n_h),
                        in0=ps_nd[:, :C].rearrange("p (h c) -> p h c", h=n_h),
                        in1=rden[:, :n_h].rearrange("p (h c) -> p h c", c=1)
                            .to_broadcast([P, n_h, c_h]),
                        op=mybir.AluOpType.mult)
                    if l + 1 < nl:
                        hb = hpool.tile([P, c_tbl_max], F16, tag="hb")
                        nc.scalar.activation(hb[:, :C], ob[:, :C],
                                             mybir.ActivationFunctionType.Relu)
                        if dbg and l == 0:
                            nc.sync.dma_start(
                                out=dbg_h1[b * P:(b + 1) * P, :],
                                in_=hb[:, :C])
                            if b == 0:
                                nc.sync.dma_start(out=dbg_lrz[:, :gG * C],
                                                  in_=lrz[:, :gG * C])
                                nc.sync.dma_start(out=dbg_xlg[:, :gG * C],
                                                  in_=xl_g[:, :gG * C])
                                nc.vector.tensor_copy(out=dbg_ex_sb[:, :gG * n_h],
                                                      in_=ex[:, :gG * n_h])
                                nc.sync.dma_start(out=dbg_ex[:, :gG * n_h],
                                                  in_=dbg_ex_sb[:, :gG * n_h])
                        # interleaved next-layer matmul for this block
                        mm_block(l + 1, wl_nxt, wr_nxt, hb[:, :C], b,
                                 do_xl=True, do_xr=True)
                    else:
                        nc.sync.dma_start(
                            out=out_d[b * P:(b + 1) * P, :],
                            in_=ob[:, :cfg.out_real])

                # allgather the next layer's xl table
                if l + 1 < nl:
                    nc.gpsimd.collective_compute(
                        "AllGather", mybir.AluOpType.bypass, replica_groups=rg,
                        ins=[xl_loc[l + 1][:, :].opt()],
                        outs=[xl_tbl[l + 1][:, :].opt()])
                if dbg and l == 0:
                    nc.sync.dma_start(out=dbg_xl0[:, :],
                                      in_=xl_tbl0[:cfg.npc, :])
                    nc.sync.dma_start(out=dbg_xl1[:, :],
                                      in_=xl_loc[1][:, :])
                    nc.sync.dma_start(out=dbg_xr1[:, :],
                                      in_=xr_sb[1][:, :cfg.nblk * 256])
    nc.compile()
    return nc


# ---------------------------------------------------------------------------
# host orchestration
# ---------------------------------------------------------------------------

def _wT_pad(w, c_tbl):
    """w: [h*oc, ic] fp32 -> [ic, c_tbl] fp16 (zero pad the out channels)."""
    w = np.asarray(w, np.float32)
    hoc, ic = w.shape
    out = np.zeros((ic, c_tbl), np.float16)
    out[:, :hoc] = w.T.astype(np.float16)
    return out


def _a_rep(a, c_tbl):
    """a: [h, oc] fp32 -> [128, c_tbl] fp16 replicated across partitions."""
    a = np.asarray(a, np.float32).reshape(-1)
    row = np.zeros(c_tbl, np.float16)
    row[:a.shape[0]] = a.astype(np.float16)
    return np.tile(row[None, :], (P, 1))


def make_in_maps(cfg, G, per_core, x, weights, meta):
    iota = np.tile(np.arange(P, dtype=np.float16)[None, :], (P, 1))
    iotac = np.tile(np.arange(P, dtype=np.float16)[:, None], (1, max(G) * P))
    x16 = np.asarray(x, np.float32).astype(np.float16)
    xpad = np.zeros((cfg.npad, cfg.layers[0]["c_in"]), np.float16)
    real = meta["perm"] >= 0
    xpad[real] = x16[meta["perm"][real]]
    xpadT = np.ascontiguousarray(xpad.T)
    shared = dict(iota=iota, iotac=iotac, x_fullT=xpadT)
    for l, L in enumerate(cfg.layers):
        wl, wr, a = weights[l]
        shared[f"w{l}l"] = _wT_pad(wl, L["c_tbl"])
        shared[f"w{l}r"] = _wT_pad(wr, L["c_cmp"])
        shared[f"a{l}"] = _a_rep(a, L["c_cmp"])
    in_maps = []
    for c in range(cfg.n_cores):
        m = dict(shared)
        m["x_ownT"] = np.ascontiguousarray(
            xpadT[:, c * cfg.npc:(c + 1) * cfg.npc])
        m.update(per_core[c])
        in_maps.append(m)
    return in_maps


_CACHE = {}


def _get_built(cfg, edge_index):
    key = hash(np.asarray(edge_index).tobytes())
    if key not in _CACHE:
        G, per_core, meta = prep_graph(cfg, edge_index)
        nc = build_nc(cfg, G)
        _CACHE[key] = (G, per_core, meta, nc)
    return _CACHE[key]


def kernel(x, edge_index,
           w1l, b1l, w1r, b1r, a1, bo1,
           w2l, b2l, w2r, b2r, a2, bo2,
           w3l, b3l, w3r, b3r, a3, bo3,
           w4l, b4l, w4r, b4r, a4, bo4,
           _trace=False):
    cfg = real_cfg()
    for b in (b1l, b1r, b2l, b2r, b3l, b3r, b4l, b4r, bo1, bo2, bo3):
        assert np.max(np.abs(np.asarray(b, np.float32))) == 0.0, \
            "non-zero internal biases not supported"
    G, per_core, meta, nc = _get_built(cfg, edge_index)
    weights = [(w1l, w1r, a1), (w2l, w2r, a2), (w3l, w3r, a3), (w4l, w4r, a4)]
    in_maps = make_in_maps(cfg, G, per_core, x, weights, meta)
    res = run_bass_kernel_spmd(nc, in_maps, core_ids=list(range(cfg.n_cores)),
                               trace=_trace)
    outs = [np.asarray(res.results[c]["out"]) for c in range(cfg.n_cores)]
    full_pos = np.concatenate(outs, axis=0).astype(np.float32)
    full = full_pos[meta["ipos"]]
    full = full + np.asarray(bo4, np.float32)[None, :]
    if _trace:
        kernel.last_exec_time_ns = res.exec_time_ns
        kernel.last_res = res
    return full


kernel.last_exec_time_ns = None
kernel.last_res = None


# revision 28
# speedup vs baseline: 1.1990x; 1.1990x over previous
"""4-layer GATv2 forward pass on 8 TRN2 NeuronCores (Bass/Tile).

Strategy (node/dst partitioning, no cross-core segment reductions):
  - Nodes are padded to 20480 and split into 8 contiguous slices of 2560
    (20 blocks of 128 dst nodes per core).  Each core owns the segment
    softmax + weighted scatter for its dst nodes, so all softmax
    reductions are core-local.
  - Edges (with self loops appended) are routed to the core/block that
    owns their dst.  Per (core, block) edge counts are padded to a
    shared multiple of 128 (G[b] groups of 128 edges) so one NEFF works
    for all 8 cores.
  - Layer 1 source transforms (xl) are computed for the FULL node table
    on every core (x is replicated), so no collective is needed before
    the first edge phase.  For layers 2-4, the matmul for layer l+1 of
    dst block b is interleaved right after the edge phase of block b
    (its input h lives in SBUF), so the AllGather of the next xl table
    can issue the moment the edge phase drains.
  - Per-edge work is edge-major (partition = edge % 128): dma_gather of
    xl[src] rows (parallel calls over the 4 SWDGE queues), xr[dst]
    expansion + z = xl+xr + leaky-relu via PE matmuls against a one-hot
    dst selector, per-head dot with `a` (DVE folds+reduce), exp, then
    the softmax denominator and alpha-weighted sum of xl[src] as one PE
    matmul per 128-edge group (SelT).
  - Softmax uses exp(logit) directly (no running max): logits are O(10)
    here, fp32 exp is exact enough, and the math is identical to the
    reference's shifted softmax.

kernel(**inputs) takes the full problem inputs and returns the full
[20000, 16] fp32 output.
"""

import numpy as np

import concourse.bass as bass
import concourse.bacc as bacc
import concourse.mybir as mybir
import concourse.tile as tile
from concourse.bass_utils import run_bass_kernel_spmd
from concourse.masks import make_identity

F16 = mybir.dt.float16
BF16 = mybir.dt.bfloat16
F32 = mybir.dt.float32
I16 = mybir.dt.int16
U8 = mybir.dt.uint8
P = 128

# model dims (fixed by the problem)
N_REAL = 20000
E_RAW = 320000
IN_CH = 128
HID = 64
HEADS = 4
OUT_CH = 16
SLOPE = 0.2

MASK_NEG = -50.0  # additive logit bias for pad edges
DEN_EPS = 1e-12   # keeps reciprocal() in range for edgeless (pad) dst rows


class Cfg:
    def __init__(self, n_cores, npc, n_real, layers, out_real):
        assert npc % P == 0
        self.n_cores = n_cores
        self.npc = npc              # nodes per core (padded)
        self.nblk = npc // P        # dst blocks per core
        self.n_real = n_real
        self.npad = n_cores * npc
        self.layers = layers        # list of dicts: c_in, c_tbl, n_h, c_h
        self.out_real = out_real    # real output channels of last layer


def real_cfg():
    layers = [
        dict(c_in=IN_CH, c_tbl=HEADS * HID, c_cmp=HEADS * HID,
             n_h=HEADS, c_h=HID),
        dict(c_in=HEADS * HID, c_tbl=HEADS * HID, c_cmp=HEADS * HID,
             n_h=HEADS, c_h=HID),
        dict(c_in=HEADS * HID, c_tbl=HEADS * HID, c_cmp=HEADS * HID,
             n_h=HEADS, c_h=HID),
        # 16 real out channels: gather table stays 128 wide (256B descriptor
        # minimum) but all edge-phase compute runs at width 32
        dict(c_in=HEADS * HID, c_tbl=P, c_cmp=32, n_h=1, c_h=32),
    ]
    return Cfg(8, 2560, N_REAL, layers, OUT_CH)


# ---------------------------------------------------------------------------
# host-side graph preprocessing
# ---------------------------------------------------------------------------

def prep_graph(cfg, edge_index):
    """Route edges (plus self loops) to (core, block) by dst; build per-core
    gather-index / dst-local / mask arrays in the exact SBUF layouts the
    kernel consumes.

    Dst nodes are REASSIGNED to (core, block) bins by LPT load balancing
    (highest in-degree first, always into the lightest non-full bin) so
    per-bin edge counts are near-uniform: the shared padded group count
    drops from ceil(max/128) to ceil(mean/128).  All device-side arrays
    are expressed in permuted "position" space; kernel() un-permutes the
    output rows at the end (meta["ipos"])."""
    n = cfg.n_real
    src = np.concatenate([np.asarray(edge_index[0], np.int64),
                          np.arange(n, dtype=np.int64)])
    dst = np.concatenate([np.asarray(edge_index[1], np.int64),
                          np.arange(n, dtype=np.int64)])
    assert src.min() >= 0 and src.max() < n and dst.min() >= 0 and dst.max() < n

    nbin = cfg.n_cores * cfg.nblk
    deg = np.bincount(dst, minlength=n)
    nodes_by_deg = np.argsort(-deg, kind="stable")
    binsum = np.zeros(nbin, np.int64)
    bincnt = np.zeros(nbin, np.int64)
    ipos = np.empty(n, np.int64)
    perm = np.full(cfg.npad, -1, np.int64)
    for v in nodes_by_deg:
        open_b = np.flatnonzero(bincnt < P)
        b = open_b[np.argmin(binsum[open_b])]
        p_ = b * P + bincnt[b]
        ipos[v] = p_
        perm[p_] = v
        binsum[b] += deg[v]
        bincnt[b] += 1
    meta = dict(perm=perm, ipos=ipos)

    src, dst = ipos[src], ipos[dst]       # positions from here on
    gblk = dst // P                       # global block id (core-major)
    order = np.argsort(gblk, kind="stable")
    src, dst, gblk = src[order], dst[order], gblk[order]

    nblk_tot = cfg.n_cores * cfg.nblk
    counts = np.bincount(gblk, minlength=nblk_tot).reshape(cfg.n_cores, cfg.nblk)
    G = np.maximum(1, (counts.max(axis=0) + P - 1) // P).astype(np.int64)  # [nblk]
    W = int(G.sum())

    # split edges per (core, block)
    starts = np.zeros(nblk_tot + 1, np.int64)
    np.cumsum(counts.reshape(-1), out=starts[1:])

    per_core = []
    for c in range(cfg.n_cores):
        xl_idx = np.zeros((P, 8 * W), np.int16)
        dloc = np.zeros((P, W), np.float16)
        dlocT = np.zeros((1, W * P), np.float16)
        mbias = np.full((P, W), MASK_NEG, np.float16)
        off = 0
        for b in range(cfg.nblk):
            gb = c * cfg.nblk + b
            s, e = starts[gb], starts[gb + 1]
            nreal = int(e - s)
            npad_e = int(G[b]) * P
            fsrc = np.zeros(npad_e, np.int64)
            fdl = np.zeros(npad_e, np.int64)
            fm = np.full(npad_e, MASK_NEG, np.float32)
            fsrc[:nreal] = src[s:e]
            fdl[:nreal] = dst[s:e] % P
            fm[:nreal] = 0.0
            # edge i -> partition i % 128, group i // 128
            dloc[:, off:off + G[b]] = fdl.reshape(G[b], P).T.astype(np.float16)
            dlocT[0, off * P:(off + int(G[b])) * P] = fdl.astype(np.float16)
            mbias[:, off:off + G[b]] = fm.reshape(G[b], P).T.astype(np.float16)
            # wrapped idx layout: wrapped[p, s] = flat[s*16 + p], replicated
            # into all 8 16-partition groups (one per GPSIMD Q7 core)
            xl_idx[:, 8 * off:8 * (off + G[b])] = np.tile(
                fsrc.astype(np.int16).reshape(-1, 16).T, (8, 1))
            off += int(G[b])
        per_core.append(dict(xl_idx=xl_idx, dloc=dloc,
                             dlocT=np.tile(dlocT, (P, 1)), mbias=mbias))
    return [int(g) for g in G], per_core, meta


# ---------------------------------------------------------------------------
# bass program
# ---------------------------------------------------------------------------

def build_nc(cfg, G):
    """Build the (single, SPMD) bass program."""
    nl = len(cfg.layers)
    W = sum(G)
    Gmax = max(G)
    c_tbl_max = max(L["c_tbl"] for L in cfg.layers)
    kc_max = max(L["c_in"] for L in cfg.layers) // P
    nblk_tbl = cfg.npad // P            # full-table blocks (layer-1 xl)

    nc = bacc.Bacc("TRN2", target_bir_lowering=False, debug=False,
                   num_devices=cfg.n_cores, num_swdge_queues=4)

    # layer-1 input, pre-transposed on the host: [c_in, nodes]
    x_fullT = nc.dram_tensor("x_fullT", [cfg.layers[0]["c_in"], cfg.npad], F16,
                             kind="ExternalInput")
    x_ownT = nc.dram_tensor("x_ownT", [cfg.layers[0]["c_in"], cfg.npc], F16,
                            kind="ExternalInput")
    xl_idx_d = nc.dram_tensor("xl_idx", [P, 8 * W], I16, kind="ExternalInput")
    dloc_d = nc.dram_tensor("dloc", [P, W], F16, kind="ExternalInput")
    dlocT_d = nc.dram_tensor("dlocT", [P, W * P], F16, kind="ExternalInput")
    iotac_d = nc.dram_tensor("iotac", [P, Gmax * P], F16, kind="ExternalInput")
    mbias_d = nc.dram_tensor("mbias", [P, W], F16, kind="ExternalInput")
    iota_d = nc.dram_tensor("iota", [P, P], F16, kind="ExternalInput")
    w_d, a_d = [], []
    for l, L in enumerate(cfg.layers):
        wl = nc.dram_tensor(f"w{l}l", [L["c_in"], L["c_tbl"]], F16,
                            kind="ExternalInput")
        wr = nc.dram_tensor(f"w{l}r", [L["c_in"], L["c_cmp"]], F16,
                            kind="ExternalInput")
        w_d.append((wl, wr))
        a_d.append(nc.dram_tensor(f"a{l}", [P, L["c_cmp"]], F16,
                                  kind="ExternalInput"))
    out_d = nc.dram_tensor("out", [cfg.npc, cfg.out_real], F32,
                           kind="ExternalOutput")
    import os
    dbg = os.environ.get("K_DEBUG") == "1"
    if dbg:
        dbg_xl0 = nc.dram_tensor("dbg_xl0", [cfg.npc, cfg.layers[0]["c_tbl"]],
                                 F16, kind="ExternalOutput")
        dbg_xl1 = nc.dram_tensor("dbg_xl1", [cfg.npc, cfg.layers[1]["c_tbl"]],
                                 F16, kind="ExternalOutput")
        dbg_xr1 = nc.dram_tensor("dbg_xr1", [P, cfg.nblk * 256], F16,
                                 kind="ExternalOutput")
        dbg_h1 = nc.dram_tensor("dbg_h1", [cfg.npc, 256], F16,
                                 kind="ExternalOutput")
        dbg_lrz = nc.dram_tensor("dbg_lrz", [P, 18 * 256], F16,
                                 kind="ExternalOutput")
        dbg_xlg = nc.dram_tensor("dbg_xlg", [P, 18 * 256], F16,
                                 kind="ExternalOutput")
        dbg_ex = nc.dram_tensor("dbg_ex", [P, 18 * HEADS], F16,
                                kind="ExternalOutput")

    rg = [list(range(cfg.n_cores))]

    with tile.TileContext(nc) as tc:
        with (
            tc.tile_pool(name="const", bufs=1) as cpool,
            tc.tile_pool(name="wts", bufs=2) as wpool,
            tc.tile_pool(name="mm", bufs=3) as mpool,
            tc.tile_pool(name="gath", bufs=4) as gpool,
            tc.tile_pool(name="gidx", bufs=16) as gipool,
            tc.tile_pool(name="edge", bufs=2) as epool,
            tc.tile_pool(name="small", bufs=2) as spool,
            tc.tile_pool(name="hbuf", bufs=3) as hpool,
            tc.tile_pool(name="xrsb", bufs=2) as xpool,
            tc.tile_pool(name="psum", bufs=2, space="PSUM") as ppool,
            tc.tile_pool(name="dram", bufs=1, space="DRAM") as dpool,
        ):
            # ---- persistent constants -------------------------------------
            iota_sb = cpool.tile([P, P], F16, tag="iota")
            nc.sync.dma_start(out=iota_sb[:], in_=iota_d[:])
            ident = cpool.tile([P, P], F16, tag="ident")
            make_identity(nc, ident[:])
            dloc_sb = cpool.tile([P, W], F16, tag="dloc")
            nc.sync.dma_start(out=dloc_sb[:], in_=dloc_d[:])
            mb_sb = cpool.tile([P, W], F16, tag="mbias")
            nc.sync.dma_start(out=mb_sb[:], in_=mbias_d[:])
            # iotac_u8[p, e] = p  (host-replicated)
            iotac_sb = cpool.tile([P, Gmax * P], F16, tag="iotacr")
            nc.sync.dma_start(out=iotac_sb[:], in_=iotac_d[:])
            if dbg:
                dbg_ex_sb = cpool.tile([P, 18 * HEADS], F16, tag="dbgex")

            # ---- per-layer DRAM scratch -----------------------------------
            xl_loc, xl_tbl = [None], [None]
            # layer 0 xl table is computed fully on every core -> Local.
            xl_tbl0 = dpool.tile([cfg.npad, cfg.layers[0]["c_tbl"]], F16,
                                 tag="xltbl0", name="xltbl0")
            for l in range(1, nl):
                L = cfg.layers[l]
                xl_loc.append(dpool.tile([cfg.npc, L["c_tbl"]], F16,
                                         tag=f"xlloc{l}", name=f"xlloc{l}"))
                xl_tbl.append(dpool.tile(
                    [cfg.npad, L["c_tbl"]], F16, tag=f"xltbl{l}",
                    name=f"xltbl{l}", addr_space="Shared"))
            xl_tbl[0] = xl_tbl0

            # per-layer xr tables stay in SBUF (written by the interleaved
            # matmul of the previous layer's edge phase)
            xr_sb = [xpool.tile([P, cfg.nblk * c_tbl_max], F16, tag="xr_sb",
                                name=f"xr_sb{l}")
                     for l in range(nl)]

            # weight tiles per layer (wpool rotates 2 buffers)
            def load_weights(l):
                L = cfg.layers[l]
                Cl, Cr, c_in = L["c_tbl"], L["c_cmp"], L["c_in"]
                kc_n = c_in // P
                wl_sb = wpool.tile([P, kc_max * c_tbl_max], F16, tag="wl")
                wr_sb = wpool.tile([P, kc_max * c_tbl_max], F16, tag="wr")
                for kc in range(kc_n):
                    nc.sync.dma_start(out=wl_sb[:, kc * Cl:(kc + 1) * Cl],
                                      in_=w_d[l][0][kc * P:(kc + 1) * P, :])
                    nc.sync.dma_start(out=wr_sb[:, kc * Cr:(kc + 1) * Cr],
                                      in_=w_d[l][1][kc * P:(kc + 1) * P, :])
                return wl_sb, wr_sb

            def load_arep(l):
                L = cfg.layers[l]
                C = L["c_cmp"]
                a_rep = wpool.tile([P, Gmax * c_tbl_max], F16, tag="arep")
                nc.sync.dma_start(
                    out=a_rep[:, :Gmax * C].rearrange("p (g c) -> p g c", g=Gmax),
                    in_=a_d[l][:].rearrange("p (g c) -> p g c", g=1)
                        .to_broadcast([P, Gmax, C]))
                return a_rep

            def mm_block(l, wl_sb, wr_sb, h_ap, blk, do_xl, do_xr):
                """Source/target transforms of one 128-node block of layer l.
                h_ap: [P, c_in] SBUF activation tile; writes xl to
                xl_loc/xl_tbl and/or xr into the resident xr_sb table."""
                L = cfg.layers[l]
                Cl, Cr, c_in = L["c_tbl"], L["c_cmp"], L["c_in"]
                kc_n = c_in // P
                hT = mpool.tile([P, kc_max * P], F16, tag="hT")
                for kc in range(kc_n):
                    pt = ppool.tile([P, P], F16, tag="pt", bufs=1)
                    nc.tensor.transpose(pt[:], h_ap[:, kc * P:(kc + 1) * P],
                                        ident[:])
                    nc.vector.tensor_copy(out=hT[:, kc * P:(kc + 1) * P],
                                          in_=pt[:])
                if do_xl:
                    ps_xl = ppool.tile([P, c_tbl_max], F32, tag="ps_mm")
                    for kc in range(kc_n):
                        nc.tensor.matmul(ps_xl[:, :Cl],
                                         lhsT=hT[:, kc * P:(kc + 1) * P],
                                         rhs=wl_sb[:, kc * Cl:(kc + 1) * Cl],
                                         start=(kc == 0), stop=(kc == kc_n - 1))
                    xl_t = mpool.tile([P, c_tbl_max], F16, tag="xl_t")
                    nc.scalar.activation(xl_t[:, :Cl], ps_xl[:, :Cl],
                                         mybir.ActivationFunctionType.Copy)
                    if l == 0:
                        nc.sync.dma_start(
                            out=xl_tbl0[blk * P:(blk + 1) * P, :],
                            in_=xl_t[:, :Cl])
                    else:
                        nc.sync.dma_start(
                            out=xl_loc[l][blk * P:(blk + 1) * P, :],
                            in_=xl_t[:, :Cl])
                if do_xr:
                    ps_xr = ppool.tile([P, c_tbl_max], F32, tag="ps_mm")
                    for kc in range(kc_n):
                        nc.tensor.matmul(ps_xr[:, :Cr],
                                         lhsT=hT[:, kc * P:(kc + 1) * P],
                                         rhs=wr_sb[:, kc * Cr:(kc + 1) * Cr],
                                         start=(kc == 0), stop=(kc == kc_n - 1))
                    nc.scalar.activation(
                        xr_sb[l][:, blk * c_tbl_max:blk * c_tbl_max + Cr],
                        ps_xr[:, :Cr], mybir.ActivationFunctionType.Copy)

            # ---- layer 0 matmul phase: full xl table, local xr ------------
            # x arrives pre-transposed, so each block's lhsT is a direct
            # slice load: no PE transposes, batched 4-block DMAs.
            wl_sb, wr_sb = load_weights(0)
            c_in0 = cfg.layers[0]["c_in"]
            C0 = cfg.layers[0]["c_tbl"]
            TB = 4
            for t0 in range(0, nblk_tbl, TB):
                xT4 = mpool.tile([P, TB * P], F16, tag="xT4")
                nc.sync.dma_start(out=xT4[:],
                                  in_=x_fullT[:, t0 * P:(t0 + TB) * P])
                xl_t4 = mpool.tile([P, TB * C0], F16, tag="xl_t4")
                for ti in range(TB):
                    ps_xl = ppool.tile([P, c_tbl_max], F32, tag="ps_mm")
                    nc.tensor.matmul(ps_xl[:, :C0],
                                     lhsT=xT4[:, ti * P:(ti + 1) * P],
                                     rhs=wl_sb[:, :C0], start=True, stop=True)
                    if ti % 2 == 0:
                        nc.scalar.activation(xl_t4[:, ti * C0:(ti + 1) * C0],
                                             ps_xl[:, :C0],
                                             mybir.ActivationFunctionType.Copy)
                    else:
                        nc.vector.tensor_copy(
                            out=xl_t4[:, ti * C0:(ti + 1) * C0],
                            in_=ps_xl[:, :C0])
                nc.sync.dma_start(
                    out=xl_tbl0[t0 * P:(t0 + TB) * P, :]
                        .rearrange("(t p) c -> p t c", p=P),
                    in_=xl_t4[:].rearrange("p (t c) -> p t c", t=TB))
            for b0 in range(0, cfg.nblk, TB):
                xT4 = mpool.tile([P, TB * P], F16, tag="xT4")
                nc.sync.dma_start(out=xT4[:],
                                  in_=x_ownT[:, b0 * P:(b0 + TB) * P])
                for bi in range(TB):
                    b = b0 + bi
                    ps_xr = ppool.tile([P, c_tbl_max], F32, tag="ps_mm")
                    nc.tensor.matmul(ps_xr[:, :C0],
                                     lhsT=xT4[:, bi * P:(bi + 1) * P],
                                     rhs=wr_sb[:, :C0], start=True, stop=True)
                    nc.scalar.activation(
                        xr_sb[0][:, b * c_tbl_max:b * c_tbl_max + C0],
                        ps_xr[:, :C0], mybir.ActivationFunctionType.Copy)

            # ---- per-layer edge phase (+ interleaved next-layer matmul) ---
            qn = [0]

            def gather_rows(tbl_ap, out_tile, off_g, n_g, C_, nm):
                """Gather n_g*128 rows from tbl_ap into out_tile
                [P, n_g, C_], split into <=6-group chunks spread over the
                4 SWDGE queues (no ordering chain: they run concurrently)."""
                CH = 6
                for k0 in range(0, n_g, CH):
                    gk = min(CH, n_g - k0)
                    it = gipool.tile([P, 8 * CH], I16, tag="idxt",
                                     name=f"idxt_{nm}_{k0}")
                    nc.sync.dma_start(
                        out=it[:, :8 * gk],
                        in_=xl_idx_d[:, 8 * (off_g + k0):
                                     8 * (off_g + k0 + gk)])
                    nc.gpsimd.dma_gather(
                        out_ap=out_tile[:, k0 * C_:(k0 + gk) * C_]
                            .rearrange("p (g c) -> p g c", c=C_),
                        in_ap=tbl_ap,
                        idxs_ap=it[:, :8 * gk],
                        num_idxs=gk * P, num_idxs_reg=gk * P,
                        elem_size=C_, queue_num=qn[0] % 4)
                    qn[0] += 1

            for l, L in enumerate(cfg.layers):
                c_in, C, n_h, c_h = L["c_in"], L["c_cmp"], L["n_h"], L["c_h"]
                CT = L["c_tbl"]
                EC = C + n_h
                a_rep = load_arep(l)
                if l + 1 < nl:
                    wl_nxt, wr_nxt = load_weights(l + 1)

                for b in range(cfg.nblk):
                    gG = G[b]
                    off = sum(G[:b])
                    xl_g = gpool.tile([P, Gmax * c_tbl_max], F16, tag="xl_g")
                    gather_rows(xl_tbl[l][:, :], xl_g, off, gG, CT,
                                f"xl{l}_{b}")
                    xl3 = xl_g[:, :gG * CT].rearrange(
                        "p (g c) -> p g c", c=CT)
                    xr_blk = xr_sb[l][:, b * c_tbl_max:b * c_tbl_max + C]
                    # Sel[d, e] = (d == dloc[e])    [for xr expansion]
                    dlt = epool.tile([P, Gmax * P], F16, tag="dlt")
                    nc.sync.dma_start(out=dlt[:, :gG * P],
                                      in_=dlocT_d[:, off * P:(off + gG) * P])
                    sel = epool.tile([P, Gmax * P], F16, tag="sel")
                    nc.vector.tensor_tensor(
                        out=sel[:, :gG * P], in0=dlt[:, :gG * P],
                        in1=iotac_sb[:, :gG * P],
                        op=mybir.AluOpType.is_equal)
                    # SelT[e, d] = (dloc[e] == d)   [for num/den matmuls]
                    selt = epool.tile([P, Gmax * P], BF16, tag="selt")
                    nc.vector.tensor_tensor(
                        out=selt[:, :gG * P].rearrange(
                            "p (g d) -> p g d", d=P),
                        in0=dloc_sb[:, off:off + gG]
                            .rearrange("p (g d) -> p g d", d=1)
                            .to_broadcast([P, gG, P]),
                        in1=iota_sb[:].rearrange("p (g d) -> p g d", g=1)
                            .to_broadcast([P, gG, P]),
                        op=mybir.AluOpType.is_equal)
                    # z (per quad of groups) in PSUM:
                    #   z_g = Sel_g^T @ xr_blk + I^T @ xl_g   -> leaky relu
                    lrz = epool.tile([P, Gmax * c_tbl_max], F16, tag="lrz")
                    for g0 in range(0, gG, 2):
                        gns = min(2, gG - g0)
                        ps_z = ppool.tile([P, 2 * c_tbl_max], F32, tag="ps_z",
                                          bufs=3)
                        # NOTE: each slice's start->stop matmul pair must stay
                        # tightly sequential; interleaving several open
                        # accumulation groups corrupts PSUM on this HW.
                        for gg in range(g0, g0 + gns):
                            sl = slice((gg - g0) * C, (gg - g0 + 1) * C)
                            nc.tensor.matmul(
                                ps_z[:, sl], lhsT=sel[:, gg * P:(gg + 1) * P],
                                rhs=xr_blk, start=True, stop=False)
                            nc.tensor.matmul(
                                ps_z[:, sl], lhsT=ident[:],
                                rhs=xl_g[:, gg * CT:gg * CT + C],
                                start=False, stop=True)
                        nc.scalar.activation(
                            lrz[:, g0 * C:(g0 + gns) * C],
                            ps_z[:, :gns * C],
                            mybir.ActivationFunctionType.Prelu,
                            alpha=SLOPE)
                    # a * LR(z)
                    alr = epool.tile([P, Gmax * c_tbl_max], F16, tag="alr")
                    nc.vector.tensor_tensor(out=alr[:, :gG * C],
                                            in0=lrz[:, :gG * C],
                                            in1=a_rep[:, :gG * C],
                                            op=mybir.AluOpType.mult)
                    # logits: two folds + reduce over c_h/4
                    ch2, ch4 = c_h // 2, c_h // 4
                    fold1 = spool.tile([P, Gmax * c_tbl_max // 2], F16,
                                       tag="fold1")
                    a4 = alr[:, :gG * C].rearrange(
                        "p (g h c) -> p g h c", h=n_h, c=c_h)
                    f13 = fold1[:, :gG * C // 2].rearrange(
                        "p (g h c) -> p g h c", h=n_h, c=ch2)
                    nc.vector.tensor_tensor(out=f13, in0=a4[:, :, :, :ch2],
                                            in1=a4[:, :, :, ch2:],
                                            op=mybir.AluOpType.add)
                    fold2 = spool.tile([P, Gmax * c_tbl_max // 4], F16,
                                       tag="fold2")
                    f23 = fold2[:, :gG * C // 4].rearrange(
                        "p (g h c) -> p g h c", h=n_h, c=ch4)
                    nc.vector.tensor_tensor(out=f23, in0=f13[:, :, :, :ch4],
                                            in1=f13[:, :, :, ch4:],
                                            op=mybir.AluOpType.add)
                    logits = spool.tile([P, Gmax * HEADS], F32, tag="logits")
                    nc.vector.tensor_reduce(
                        out=logits[:, :gG * n_h].rearrange(
                            "p (g h) -> p g h", h=n_h),
                        in_=f23,
                        axis=mybir.AxisListType.X, op=mybir.AluOpType.add)
                    # pad-edge mask as additive bias
                    logm = spool.tile([P, Gmax * HEADS], F32, tag="logm")
                    nc.vector.tensor_tensor(
                        out=logm[:, :gG * n_h].rearrange(
                            "p (g h) -> p g h", h=n_h),
                        in0=logits[:, :gG * n_h].rearrange(
                            "p (g h) -> p g h", h=n_h),
                        in1=mb_sb[:, off:off + gG]
                            .rearrange("p (g h) -> p g h", h=1)
                            .to_broadcast([P, gG, n_h]),
                        op=mybir.AluOpType.add)
                    ex = spool.tile([P, Gmax * HEADS], BF16, tag="ex")
                    nc.scalar.activation(ex[:, :gG * n_h], logm[:, :gG * n_h],
                                         mybir.ActivationFunctionType.Exp)
                    # edata = [ex * xl[src] | ex]  (ex broadcast over c_h)
                    edata = epool.tile([P, Gmax * (c_tbl_max + HEADS)], BF16,
                                       tag="edata")
                    ed3 = edata[:, :gG * EC].rearrange("p (g c) -> p g c", c=EC)
                    nc.vector.tensor_tensor(
                        out=ed3[:, :, :C].rearrange(
                            "p g (h c) -> p g h c", c=c_h),
                        in0=xl3[:, :, :C].rearrange(
                            "p g (h c) -> p g h c", c=c_h),
                        in1=ex[:, :gG * n_h].rearrange(
                            "p (g h c) -> p g h c", h=n_h, c=1)
                            .to_broadcast([P, gG, n_h, c_h]),
                        op=mybir.AluOpType.mult)
                    nc.scalar.activation(
                        ed3[:, :, C:],
                        ex[:, :gG * n_h].rearrange("p (g h) -> p g h",
                                                   h=n_h),
                        mybir.ActivationFunctionType.Copy)
                    # segment sums via PE: psum[d, :] += SelT_g^T @ edata_g
                    ps_nd = ppool.tile([P, EC], F32, tag="ps_nd", bufs=2)
                    for g in range(gG):
                        nc.tensor.matmul(
                            ps_nd[:], lhsT=selt[:, g * P:(g + 1) * P],
                            rhs=edata[:, g * EC:(g + 1) * EC],
                            start=(g == 0), stop=(g == gG - 1))
                    den_e = spool.tile([P, HEADS], F32, tag="den_e")
                    nc.vector.tensor_scalar(
                        out=den_e[:, :n_h], in0=ps_nd[:, C:], scalar1=DEN_EPS,
                        scalar2=None, op0=mybir.AluOpType.add)
                    rden = spool.tile([P, HEADS], F32, tag="rden")
                    nc.vector.reciprocal(rden[:, :n_h], den_e[:, :n_h])
                    ob = spool.tile([P, c_tbl_max], F32, tag="ob")
                    nc.vector.tensor_tensor(
                        out=ob[:, :C].rearrange("p (h c) -> p h c", h=n_h),
                        in0=ps_nd[:, :C].rearrange("p (h c) -> p h c", h=n_h),
                        in1=rden[:, :n_h].rearrange("p (h c) -> p h c", c=1)
                            .to_broadcast([P, n_h, c_h]),
                        op=mybir.AluOpType.mult)
                    if l + 1 < nl:
                        hb = hpool.tile([P, c_tbl_max], F16, tag="hb")
                        nc.scalar.activation(hb[:, :C], ob[:, :C],
                                             mybir.ActivationFunctionType.Relu)
                        if dbg and l == 0:
                            nc.sync.dma_start(
                                out=dbg_h1[b * P:(b + 1) * P, :],
                                in_=hb[:, :C])
                            if b == 0:
                                nc.sync.dma_start(out=dbg_lrz[:, :gG * C],
                                                  in_=lrz[:, :gG * C])
                                nc.sync.dma_start(out=dbg_xlg[:, :gG * C],
                                                  in_=xl_g[:, :gG * C])
                                nc.vector.tensor_copy(out=dbg_ex_sb[:, :gG * n_h],
                                                      in_=ex[:, :gG * n_h])
                                nc.sync.dma_start(out=dbg_ex[:, :gG * n_h],
                                                  in_=dbg_ex_sb[:, :gG * n_h])
                        # interleaved next-layer matmul for this block
                        mm_block(l + 1, wl_nxt, wr_nxt, hb[:, :C], b,
                                 do_xl=True, do_xr=True)
                    else:
                        nc.sync.dma_start(
                            out=out_d[b * P:(b + 1) * P, :],
                            in_=ob[:, :cfg.out_real])

                # allgather the next layer's xl table
                if l + 1 < nl:
                    nc.gpsimd.collective_compute(
                        "AllGather", mybir.AluOpType.bypass, replica_groups=rg,
                        ins=[xl_loc[l + 1][:, :].opt()],
                        outs=[xl_tbl[l + 1][:, :].opt()])
                if dbg and l == 0:
                    nc.sync.dma_start(out=dbg_xl0[:, :],
                                      in_=xl_tbl0[:cfg.npc, :])
                    nc.sync.dma_start(out=dbg_xl1[:, :],
                                      in_=xl_loc[1][:, :])
                    nc.sync.dma_start(out=dbg_xr1[:, :],
                                      in_=xr_sb[1][:, :cfg.nblk * 256])
    nc.compile()
    return nc


# ---------------------------------------------------------------------------
# host orchestration
# ---------------------------------------------------------------------------

def _wT_pad(w, c_tbl):
    """w: [h*oc, ic] fp32 -> [ic, c_tbl] fp16 (zero pad the out channels)."""
    w = np.asarray(w, np.float32)
    hoc, ic = w.shape
    out = np.zeros((ic, c_tbl), np.float16)
    out[:, :hoc] = w.T.astype(np.float16)
    return out


def _a_rep(a, c_tbl):
    """a: [h, oc] fp32 -> [128, c_tbl] fp16 replicated across partitions."""
    a = np.asarray(a, np.float32).reshape(-1)
    row = np.zeros(c_tbl, np.float16)
    row[:a.shape[0]] = a.astype(np.float16)
    return np.tile(row[None, :], (P, 1))


def make_in_maps(cfg, G, per_core, x, weights, meta):
    iota = np.tile(np.arange(P, dtype=np.float16)[None, :], (P, 1))
    iotac = np.tile(np.arange(P, dtype=np.float16)[:, None], (1, max(G) * P))
    x16 = np.asarray(x, np.float32).astype(np.float16)
    xpad = np.zeros((cfg.npad, cfg.layers[0]["c_in"]), np.float16)
    real = meta["perm"] >= 0
    xpad[real] = x16[meta["perm"][real]]
    xpadT = np.ascontiguousarray(xpad.T)
    shared = dict(iota=iota, iotac=iotac, x_fullT=xpadT)
    for l, L in enumerate(cfg.layers):
        wl, wr, a = weights[l]
        shared[f"w{l}l"] = _wT_pad(wl, L["c_tbl"])
        shared[f"w{l}r"] = _wT_pad(wr, L["c_cmp"])
        shared[f"a{l}"] = _a_rep(a, L["c_cmp"])
    in_maps = []
    for c in range(cfg.n_cores):
        m = dict(shared)
        m["x_ownT"] = np.ascontiguousarray(
            xpadT[:, c * cfg.npc:(c + 1) * cfg.npc])
        m.update(per_core[c])
        in_maps.append(m)
    return in_maps


_CACHE = {}


def _get_built(cfg, edge_index):
    key = hash(np.asarray(edge_index).tobytes())
    if key not in _CACHE:
        G, per_core, meta = prep_graph(cfg, edge_index)
        nc = build_nc(cfg, G)
        _CACHE[key] = (G, per_core, meta, nc)
    return _CACHE[key]


def kernel(x, edge_index,
           w1l, b1l, w1r, b1r, a1, bo1,
           w2l, b2l, w2r, b2r, a2, bo2,
           w3l, b3l, w3r, b3r, a3, bo3,
           w4l, b4l, w4r, b4r, a4, bo4,
           _trace=False):
    cfg = real_cfg()
    for b in (b1l, b1r, b2l, b2r, b3l, b3r, b4l, b4r, bo1, bo2, bo3):
        assert np.max(np.abs(np.asarray(b, np.float32))) == 0.0, \
            "non-zero internal biases not supported"
    G, per_core, meta, nc = _get_built(cfg, edge_index)
    weights = [(w1l, w1r, a1), (w2l, w2r, a2), (w3l, w3r, a3), (w4l, w4r, a4)]
    in_maps = make_in_maps(cfg, G, per_core, x, weights, meta)
    res = run_bass_kernel_spmd(nc, in_maps, core_ids=list(range(cfg.n_cores)),
                               trace=_trace)
    outs = [np.asarray(res.results[c]["out"]) for c in range(cfg.n_cores)]
    full_pos = np.concatenate(outs, axis=0).astype(np.float32)
    full = full_pos[meta["ipos"]]
    full = full + np.asarray(bo4, np.float32)[None, :]
    if _trace:
        kernel.last_exec_time_ns = res.exec_time_ns
        kernel.last_res = res
    return full


kernel.last_exec_time_ns = None
kernel.last_res = None


# revision 29
# speedup vs baseline: 1.2477x; 1.0406x over previous
"""4-layer GATv2 forward pass on 8 TRN2 NeuronCores (Bass/Tile).

Strategy (node/dst partitioning, no cross-core segment reductions):
  - Nodes are padded to 20480 and split into 8 contiguous slices of 2560
    (20 blocks of 128 dst nodes per core).  Each core owns the segment
    softmax + weighted scatter for its dst nodes, so all softmax
    reductions are core-local.
  - Edges (with self loops appended) are routed to the core/block that
    owns their dst.  Per (core, block) edge counts are padded to a
    shared multiple of 128 (G[b] groups of 128 edges) so one NEFF works
    for all 8 cores.
  - Layer 1 source transforms (xl) are computed for the FULL node table
    on every core (x is replicated), so no collective is needed before
    the first edge phase.  For layers 2-4, the matmul for layer l+1 of
    dst block b is interleaved right after the edge phase of block b
    (its input h lives in SBUF), so the AllGather of the next xl table
    can issue the moment the edge phase drains.
  - Per-edge work is edge-major (partition = edge % 128): dma_gather of
    xl[src] rows (parallel calls over the 4 SWDGE queues), xr[dst]
    expansion + z = xl+xr + leaky-relu via PE matmuls against a one-hot
    dst selector, per-head dot with `a` (DVE folds+reduce), exp, then
    the softmax denominator and alpha-weighted sum of xl[src] as one PE
    matmul per 128-edge group (SelT).
  - Softmax uses exp(logit) directly (no running max): logits are O(10)
    here, fp32 exp is exact enough, and the math is identical to the
    reference's shifted softmax.

kernel(**inputs) takes the full problem inputs and returns the full
[20000, 16] fp32 output.
"""

import numpy as np

import concourse.bass as bass
import concourse.bacc as bacc
import concourse.mybir as mybir
import concourse.tile as tile
from concourse.bass_utils import run_bass_kernel_spmd
from concourse.masks import make_identity

F16 = mybir.dt.float16
BF16 = mybir.dt.bfloat16
F32 = mybir.dt.float32
I16 = mybir.dt.int16
U8 = mybir.dt.uint8
P = 128

# model dims (fixed by the problem)
N_REAL = 20000
E_RAW = 320000
IN_CH = 128
HID = 64
HEADS = 4
OUT_CH = 16
SLOPE = 0.2

MASK_NEG = -50.0  # additive logit bias for pad edges
DEN_EPS = 1e-12   # keeps reciprocal() in range for edgeless (pad) dst rows


class Cfg:
    def __init__(self, n_cores, npc, n_real, layers, out_real):
        assert npc % P == 0
        self.n_cores = n_cores
        self.npc = npc              # nodes per core (padded)
        self.nblk = npc // P        # dst blocks per core
        self.n_real = n_real
        self.npad = n_cores * npc
        self.layers = layers        # list of dicts: c_in, c_tbl, n_h, c_h
        self.out_real = out_real    # real output channels of last layer


def real_cfg():
    layers = [
        dict(c_in=IN_CH, c_tbl=HEADS * HID, c_cmp=HEADS * HID,
             n_h=HEADS, c_h=HID),
        dict(c_in=HEADS * HID, c_tbl=HEADS * HID, c_cmp=HEADS * HID,
             n_h=HEADS, c_h=HID),
        dict(c_in=HEADS * HID, c_tbl=HEADS * HID, c_cmp=HEADS * HID,
             n_h=HEADS, c_h=HID),
        # 16 real out channels: gather table stays 128 wide (256B descriptor
        # minimum) but all edge-phase compute runs at width 32
        dict(c_in=HEADS * HID, c_tbl=P, c_cmp=32, n_h=1, c_h=32),
    ]
    return Cfg(8, 2560, N_REAL, layers, OUT_CH)


# ---------------------------------------------------------------------------
# host-side graph preprocessing
# ---------------------------------------------------------------------------

def prep_graph(cfg, edge_index):
    """Route edges (plus self loops) to (core, block) by dst; build per-core
    gather-index / dst-local / mask arrays in the exact SBUF layouts the
    kernel consumes.

    Dst nodes are REASSIGNED to (core, block) bins by LPT load balancing
    (highest in-degree first, always into the lightest non-full bin) so
    per-bin edge counts are near-uniform: the shared padded group count
    drops from ceil(max/128) to ceil(mean/128).  All device-side arrays
    are expressed in permuted "position" space; kernel() un-permutes the
    output rows at the end (meta["ipos"])."""
    n = cfg.n_real
    src = np.concatenate([np.asarray(edge_index[0], np.int64),
                          np.arange(n, dtype=np.int64)])
    dst = np.concatenate([np.asarray(edge_index[1], np.int64),
                          np.arange(n, dtype=np.int64)])
    assert src.min() >= 0 and src.max() < n and dst.min() >= 0 and dst.max() < n

    nbin = cfg.n_cores * cfg.nblk
    deg = np.bincount(dst, minlength=n)
    nodes_by_deg = np.argsort(-deg, kind="stable")
    binsum = np.zeros(nbin, np.int64)
    bincnt = np.zeros(nbin, np.int64)
    ipos = np.empty(n, np.int64)
    perm = np.full(cfg.npad, -1, np.int64)
    for v in nodes_by_deg:
        open_b = np.flatnonzero(bincnt < P)
        b = open_b[np.argmin(binsum[open_b])]
        p_ = b * P + bincnt[b]
        ipos[v] = p_
        perm[p_] = v
        binsum[b] += deg[v]
        bincnt[b] += 1
    meta = dict(perm=perm, ipos=ipos)

    src, dst = ipos[src], ipos[dst]       # positions from here on
    gblk = dst // P                       # global block id (core-major)
    order = np.argsort(gblk, kind="stable")
    src, dst, gblk = src[order], dst[order], gblk[order]

    nblk_tot = cfg.n_cores * cfg.nblk
    counts = np.bincount(gblk, minlength=nblk_tot).reshape(cfg.n_cores, cfg.nblk)
    G = np.maximum(1, (counts.max(axis=0) + P - 1) // P).astype(np.int64)  # [nblk]
    W = int(G.sum())

    # split edges per (core, block)
    starts = np.zeros(nblk_tot + 1, np.int64)
    np.cumsum(counts.reshape(-1), out=starts[1:])

    per_core = []
    for c in range(cfg.n_cores):
        xl_idx = np.zeros((P, 8 * W), np.int16)
        dloc = np.zeros((P, W), np.float16)
        dlocT = np.zeros((1, W * P), np.float16)
        mbias = np.full((P, W), MASK_NEG, np.float16)
        off = 0
        for b in range(cfg.nblk):
            gb = c * cfg.nblk + b
            s, e = starts[gb], starts[gb + 1]
            nreal = int(e - s)
            npad_e = int(G[b]) * P
            fsrc = np.zeros(npad_e, np.int64)
            fdl = np.zeros(npad_e, np.int64)
            fm = np.full(npad_e, MASK_NEG, np.float32)
            fsrc[:nreal] = src[s:e]
            fdl[:nreal] = dst[s:e] % P
            fm[:nreal] = 0.0
            # edge i -> partition i % 128, group i // 128
            dloc[:, off:off + G[b]] = fdl.reshape(G[b], P).T.astype(np.float16)
            dlocT[0, off * P:(off + int(G[b])) * P] = fdl.astype(np.float16)
            mbias[:, off:off + G[b]] = fm.reshape(G[b], P).T.astype(np.float16)
            # wrapped idx layout: wrapped[p, s] = flat[s*16 + p], replicated
            # into all 8 16-partition groups (one per GPSIMD Q7 core)
            xl_idx[:, 8 * off:8 * (off + G[b])] = np.tile(
                fsrc.astype(np.int16).reshape(-1, 16).T, (8, 1))
            off += int(G[b])
        per_core.append(dict(xl_idx=xl_idx, dloc=dloc,
                             dlocT=np.tile(dlocT, (P, 1)), mbias=mbias))
    return [int(g) for g in G], per_core, meta


# ---------------------------------------------------------------------------
# bass program
# ---------------------------------------------------------------------------

def build_nc(cfg, G):
    """Build the (single, SPMD) bass program."""
    nl = len(cfg.layers)
    W = sum(G)
    Gmax = max(G)
    c_tbl_max = max(L["c_tbl"] for L in cfg.layers)
    kc_max = max(L["c_in"] for L in cfg.layers) // P
    nblk_tbl = cfg.npad // P            # full-table blocks (layer-1 xl)

    nc = bacc.Bacc("TRN2", target_bir_lowering=False, debug=False,
                   num_devices=cfg.n_cores, num_swdge_queues=4)

    # layer-1 input, pre-transposed on the host: [c_in, nodes]
    x_fullT = nc.dram_tensor("x_fullT", [cfg.layers[0]["c_in"], cfg.npad], F16,
                             kind="ExternalInput")
    x_ownT = nc.dram_tensor("x_ownT", [cfg.layers[0]["c_in"], cfg.npc], F16,
                            kind="ExternalInput")
    xl_idx_d = nc.dram_tensor("xl_idx", [P, 8 * W], I16, kind="ExternalInput")
    dloc_d = nc.dram_tensor("dloc", [P, W], F16, kind="ExternalInput")
    dlocT_d = nc.dram_tensor("dlocT", [P, W * P], F16, kind="ExternalInput")
    iotac_d = nc.dram_tensor("iotac", [P, Gmax * P], F16, kind="ExternalInput")
    mbias_d = nc.dram_tensor("mbias", [P, W], F16, kind="ExternalInput")
    iota_d = nc.dram_tensor("iota", [P, P], F16, kind="ExternalInput")
    w_d, a_d = [], []
    for l, L in enumerate(cfg.layers):
        wl = nc.dram_tensor(f"w{l}l", [L["c_in"], L["c_tbl"]], F16,
                            kind="ExternalInput")
        wr = nc.dram_tensor(f"w{l}r", [L["c_in"], L["c_cmp"]], F16,
                            kind="ExternalInput")
        w_d.append((wl, wr))
        a_d.append(nc.dram_tensor(f"a{l}", [P, L["c_cmp"]], F16,
                                  kind="ExternalInput"))
    out_d = nc.dram_tensor("out", [cfg.npc, cfg.out_real], F32,
                           kind="ExternalOutput")
    import os
    dbg = os.environ.get("K_DEBUG") == "1"
    if dbg:
        dbg_xl0 = nc.dram_tensor("dbg_xl0", [cfg.npc, cfg.layers[0]["c_tbl"]],
                                 F16, kind="ExternalOutput")
        dbg_xl1 = nc.dram_tensor("dbg_xl1", [cfg.npc, cfg.layers[1]["c_tbl"]],
                                 F16, kind="ExternalOutput")
        dbg_xr1 = nc.dram_tensor("dbg_xr1", [P, cfg.nblk * 256], F16,
                                 kind="ExternalOutput")
        dbg_h1 = nc.dram_tensor("dbg_h1", [cfg.npc, 256], F16,
                                 kind="ExternalOutput")
        dbg_lrz = nc.dram_tensor("dbg_lrz", [P, 18 * 256], F16,
                                 kind="ExternalOutput")
        dbg_xlg = nc.dram_tensor("dbg_xlg", [P, 18 * 256], F16,
                                 kind="ExternalOutput")
        dbg_ex = nc.dram_tensor("dbg_ex", [P, 18 * HEADS], F16,
                                kind="ExternalOutput")

    rg = [list(range(cfg.n_cores))]

    with tile.TileContext(nc) as tc:
        with (
            tc.tile_pool(name="const", bufs=1) as cpool,
            tc.tile_pool(name="wts", bufs=2) as wpool,
            tc.tile_pool(name="mm", bufs=3) as mpool,
            tc.tile_pool(name="gath", bufs=4) as gpool,
            tc.tile_pool(name="gidx", bufs=16) as gipool,
            tc.tile_pool(name="edge", bufs=2) as epool,
            tc.tile_pool(name="small", bufs=2) as spool,
            tc.tile_pool(name="hbuf", bufs=3) as hpool,
            tc.tile_pool(name="xrsb", bufs=2) as xpool,
            tc.tile_pool(name="psum", bufs=2, space="PSUM") as ppool,
            tc.tile_pool(name="dram", bufs=1, space="DRAM") as dpool,
        ):
            # ---- persistent constants -------------------------------------
            iota_sb = cpool.tile([P, P], F16, tag="iota")
            nc.sync.dma_start(out=iota_sb[:], in_=iota_d[:])
            ident = cpool.tile([P, P], F16, tag="ident")
            make_identity(nc, ident[:])
            dloc_sb = cpool.tile([P, W], F16, tag="dloc")
            nc.sync.dma_start(out=dloc_sb[:], in_=dloc_d[:])
            mb_sb = cpool.tile([P, W], F16, tag="mbias")
            nc.sync.dma_start(out=mb_sb[:], in_=mbias_d[:])
            # iotac_u8[p, e] = p  (host-replicated)
            iotac_sb = cpool.tile([P, Gmax * P], F16, tag="iotacr")
            nc.sync.dma_start(out=iotac_sb[:], in_=iotac_d[:])
            if dbg:
                dbg_ex_sb = cpool.tile([P, 18 * HEADS], F16, tag="dbgex")

            # ---- per-layer DRAM scratch -----------------------------------
            xl_loc, xl_tbl = [None], [None]
            # layer 0 xl table is computed fully on every core -> Local.
            xl_tbl0 = dpool.tile([cfg.npad, cfg.layers[0]["c_tbl"]], F16,
                                 tag="xltbl0", name="xltbl0")
            for l in range(1, nl):
                L = cfg.layers[l]
                xl_loc.append(dpool.tile([cfg.npc, L["c_tbl"]], F16,
                                         tag=f"xlloc{l}", name=f"xlloc{l}"))
                xl_tbl.append(dpool.tile(
                    [cfg.npad, L["c_tbl"]], F16, tag=f"xltbl{l}",
                    name=f"xltbl{l}", addr_space="Shared"))
            xl_tbl[0] = xl_tbl0

            # per-layer xr tables stay in SBUF (written by the interleaved
            # matmul of the previous layer's edge phase)
            xr_sb = [xpool.tile([P, cfg.nblk * c_tbl_max], F16, tag="xr_sb",
                                name=f"xr_sb{l}")
                     for l in range(nl)]

            # weight tiles per layer (wpool rotates 2 buffers)
            def load_weights(l):
                L = cfg.layers[l]
                Cl, Cr, c_in = L["c_tbl"], L["c_cmp"], L["c_in"]
                kc_n = c_in // P
                wl_sb = wpool.tile([P, kc_max * c_tbl_max], F16, tag="wl")
                wr_sb = wpool.tile([P, kc_max * c_tbl_max], F16, tag="wr")
                for kc in range(kc_n):
                    nc.sync.dma_start(out=wl_sb[:, kc * Cl:(kc + 1) * Cl],
                                      in_=w_d[l][0][kc * P:(kc + 1) * P, :])
                    nc.sync.dma_start(out=wr_sb[:, kc * Cr:(kc + 1) * Cr],
                                      in_=w_d[l][1][kc * P:(kc + 1) * P, :])
                return wl_sb, wr_sb

            def load_arep(l):
                L = cfg.layers[l]
                C = L["c_cmp"]
                a_rep = wpool.tile([P, Gmax * c_tbl_max], F16, tag="arep")
                nc.sync.dma_start(
                    out=a_rep[:, :Gmax * C].rearrange("p (g c) -> p g c", g=Gmax),
                    in_=a_d[l][:].rearrange("p (g c) -> p g c", g=1)
                        .to_broadcast([P, Gmax, C]))
                return a_rep

            def mm_block(l, wl_sb, wr_sb, h_ap, blk, do_xl, do_xr):
                """Source/target transforms of one 128-node block of layer l.
                h_ap: [P, c_in] SBUF activation tile; writes xl to
                xl_loc/xl_tbl and/or xr into the resident xr_sb table."""
                L = cfg.layers[l]
                Cl, Cr, c_in = L["c_tbl"], L["c_cmp"], L["c_in"]
                kc_n = c_in // P
                hT = mpool.tile([P, kc_max * P], F16, tag="hT")
                for kc in range(kc_n):
                    pt = ppool.tile([P, P], F16, tag="pt", bufs=1)
                    nc.tensor.transpose(pt[:], h_ap[:, kc * P:(kc + 1) * P],
                                        ident[:])
                    nc.vector.tensor_copy(out=hT[:, kc * P:(kc + 1) * P],
                                          in_=pt[:])
                if do_xl:
                    ps_xl = ppool.tile([P, c_tbl_max], F32, tag="ps_mm")
                    for kc in range(kc_n):
                        nc.tensor.matmul(ps_xl[:, :Cl],
                                         lhsT=hT[:, kc * P:(kc + 1) * P],
                                         rhs=wl_sb[:, kc * Cl:(kc + 1) * Cl],
                                         start=(kc == 0), stop=(kc == kc_n - 1))
                    xl_t = mpool.tile([P, c_tbl_max], F16, tag="xl_t")
                    nc.scalar.activation(xl_t[:, :Cl], ps_xl[:, :Cl],
                                         mybir.ActivationFunctionType.Copy)
                    if l == 0:
                        nc.sync.dma_start(
                            out=xl_tbl0[blk * P:(blk + 1) * P, :],
                            in_=xl_t[:, :Cl])
                    else:
                        nc.sync.dma_start(
                            out=xl_loc[l][blk * P:(blk + 1) * P, :],
                            in_=xl_t[:, :Cl])
                if do_xr:
                    ps_xr = ppool.tile([P, c_tbl_max], F32, tag="ps_mm")
                    for kc in range(kc_n):
                        nc.tensor.matmul(ps_xr[:, :Cr],
                                         lhsT=hT[:, kc * P:(kc + 1) * P],
                                         rhs=wr_sb[:, kc * Cr:(kc + 1) * Cr],
                                         start=(kc == 0), stop=(kc == kc_n - 1))
                    nc.scalar.activation(
                        xr_sb[l][:, blk * c_tbl_max:blk * c_tbl_max + Cr],
                        ps_xr[:, :Cr], mybir.ActivationFunctionType.Copy)

            # ---- layer 0 matmul phase: full xl table, local xr ------------
            # x arrives pre-transposed, so each block's lhsT is a direct
            # slice load: no PE transposes, batched 4-block DMAs.
            wl_sb, wr_sb = load_weights(0)
            c_in0 = cfg.layers[0]["c_in"]
            C0 = cfg.layers[0]["c_tbl"]
            TB = 4
            for t0 in range(0, nblk_tbl, TB):
                xT4 = mpool.tile([P, TB * P], F16, tag="xT4")
                nc.sync.dma_start(out=xT4[:],
                                  in_=x_fullT[:, t0 * P:(t0 + TB) * P])
                xl_t4 = mpool.tile([P, TB * C0], F16, tag="xl_t4")
                for ti in range(TB):
                    ps_xl = ppool.tile([P, c_tbl_max], F32, tag="ps_mm")
                    nc.tensor.matmul(ps_xl[:, :C0],
                                     lhsT=xT4[:, ti * P:(ti + 1) * P],
                                     rhs=wl_sb[:, :C0], start=True, stop=True)
                    if ti % 2 == 0:
                        nc.scalar.activation(xl_t4[:, ti * C0:(ti + 1) * C0],
                                             ps_xl[:, :C0],
                                             mybir.ActivationFunctionType.Copy)
                    else:
                        nc.vector.tensor_copy(
                            out=xl_t4[:, ti * C0:(ti + 1) * C0],
                            in_=ps_xl[:, :C0])
                nc.sync.dma_start(
                    out=xl_tbl0[t0 * P:(t0 + TB) * P, :]
                        .rearrange("(t p) c -> p t c", p=P),
                    in_=xl_t4[:].rearrange("p (t c) -> p t c", t=TB))
            for b0 in range(0, cfg.nblk, TB):
                xT4 = mpool.tile([P, TB * P], F16, tag="xT4")
                nc.sync.dma_start(out=xT4[:],
                                  in_=x_ownT[:, b0 * P:(b0 + TB) * P])
                for bi in range(TB):
                    b = b0 + bi
                    ps_xr = ppool.tile([P, c_tbl_max], F32, tag="ps_mm")
                    nc.tensor.matmul(ps_xr[:, :C0],
                                     lhsT=xT4[:, bi * P:(bi + 1) * P],
                                     rhs=wr_sb[:, :C0], start=True, stop=True)
                    nc.scalar.activation(
                        xr_sb[0][:, b * c_tbl_max:b * c_tbl_max + C0],
                        ps_xr[:, :C0], mybir.ActivationFunctionType.Copy)

            # ---- per-layer edge phase (+ interleaved next-layer matmul) ---
            qn = [0]

            def gather_rows(tbl_ap, out_tile, off_g, n_g, C_, nm):
                """Gather n_g*128 rows from tbl_ap into out_tile
                [P, n_g, C_], split into <=6-group chunks spread over the
                4 SWDGE queues (no ordering chain: they run concurrently)."""
                CH = 5
                for k0 in range(0, n_g, CH):
                    gk = min(CH, n_g - k0)
                    it = gipool.tile([P, 8 * CH], I16, tag="idxt",
                                     name=f"idxt_{nm}_{k0}")
                    nc.sync.dma_start(
                        out=it[:, :8 * gk],
                        in_=xl_idx_d[:, 8 * (off_g + k0):
                                     8 * (off_g + k0 + gk)])
                    nc.gpsimd.dma_gather(
                        out_ap=out_tile[:, k0 * C_:(k0 + gk) * C_]
                            .rearrange("p (g c) -> p g c", c=C_),
                        in_ap=tbl_ap,
                        idxs_ap=it[:, :8 * gk],
                        num_idxs=gk * P, num_idxs_reg=gk * P,
                        elem_size=C_, queue_num=qn[0] % 4)
                    qn[0] += 1

            for l, L in enumerate(cfg.layers):
                c_in, C, n_h, c_h = L["c_in"], L["c_cmp"], L["n_h"], L["c_h"]
                CT = L["c_tbl"]
                EC = C + n_h
                a_rep = load_arep(l)
                if l + 1 < nl:
                    wl_nxt, wr_nxt = load_weights(l + 1)

                for b in range(cfg.nblk):
                    gG = G[b]
                    off = sum(G[:b])
                    xl_g = gpool.tile([P, Gmax * c_tbl_max], F16, tag="xl_g")
                    gather_rows(xl_tbl[l][:, :], xl_g, off, gG, CT,
                                f"xl{l}_{b}")
                    xl3 = xl_g[:, :gG * CT].rearrange(
                        "p (g c) -> p g c", c=CT)
                    xr_blk = xr_sb[l][:, b * c_tbl_max:b * c_tbl_max + C]
                    # Sel[d, e] = (d == dloc[e])    [for xr expansion]
                    dlt = epool.tile([P, Gmax * P], F16, tag="dlt")
                    nc.sync.dma_start(out=dlt[:, :gG * P],
                                      in_=dlocT_d[:, off * P:(off + gG) * P])
                    sel = epool.tile([P, Gmax * P], F16, tag="sel")
                    nc.vector.tensor_tensor(
                        out=sel[:, :gG * P], in0=dlt[:, :gG * P],
                        in1=iotac_sb[:, :gG * P],
                        op=mybir.AluOpType.is_equal)
                    # SelT[e, d] = (dloc[e] == d)   [for num/den matmuls]
                    selt = epool.tile([P, Gmax * P], BF16, tag="selt")
                    nc.vector.tensor_tensor(
                        out=selt[:, :gG * P].rearrange(
                            "p (g d) -> p g d", d=P),
                        in0=dloc_sb[:, off:off + gG]
                            .rearrange("p (g d) -> p g d", d=1)
                            .to_broadcast([P, gG, P]),
                        in1=iota_sb[:].rearrange("p (g d) -> p g d", g=1)
                            .to_broadcast([P, gG, P]),
                        op=mybir.AluOpType.is_equal)
                    # z (per quad of groups) in PSUM:
                    #   z_g = Sel_g^T @ xr_blk + I^T @ xl_g   -> leaky relu
                    lrz = epool.tile([P, Gmax * c_tbl_max], F16, tag="lrz")
                    for g0 in range(0, gG, 2):
                        gns = min(2, gG - g0)
                        ps_z = ppool.tile([P, 2 * c_tbl_max], F32, tag="ps_z",
                                          bufs=3)
                        # NOTE: each slice's start->stop matmul pair must stay
                        # tightly sequential; interleaving several open
                        # accumulation groups corrupts PSUM on this HW.
                        for gg in range(g0, g0 + gns):
                            sl = slice((gg - g0) * C, (gg - g0 + 1) * C)
                            nc.tensor.matmul(
                                ps_z[:, sl], lhsT=sel[:, gg * P:(gg + 1) * P],
                                rhs=xr_blk, start=True, stop=False)
                            nc.tensor.matmul(
                                ps_z[:, sl], lhsT=ident[:],
                                rhs=xl_g[:, gg * CT:gg * CT + C],
                                start=False, stop=True)
                        nc.scalar.activation(
                            lrz[:, g0 * C:(g0 + gns) * C],
                            ps_z[:, :gns * C],
                            mybir.ActivationFunctionType.Prelu,
                            alpha=SLOPE)
                    # a * LR(z)
                    alr = epool.tile([P, Gmax * c_tbl_max], F16, tag="alr")
                    nc.vector.tensor_tensor(out=alr[:, :gG * C],
                                            in0=lrz[:, :gG * C],
                                            in1=a_rep[:, :gG * C],
                                            op=mybir.AluOpType.mult)
                    # logits: two folds + reduce over c_h/4
                    ch2, ch4 = c_h // 2, c_h // 4
                    fold1 = spool.tile([P, Gmax * c_tbl_max // 2], F16,
                                       tag="fold1")
                    a4 = alr[:, :gG * C].rearrange(
                        "p (g h c) -> p g h c", h=n_h, c=c_h)
                    f13 = fold1[:, :gG * C // 2].rearrange(
                        "p (g h c) -> p g h c", h=n_h, c=ch2)
                    nc.vector.tensor_tensor(out=f13, in0=a4[:, :, :, :ch2],
                                            in1=a4[:, :, :, ch2:],
                                            op=mybir.AluOpType.add)
                    fold2 = spool.tile([P, Gmax * c_tbl_max // 4], F16,
                                       tag="fold2")
                    f23 = fold2[:, :gG * C // 4].rearrange(
                        "p (g h c) -> p g h c", h=n_h, c=ch4)
                    nc.vector.tensor_tensor(out=f23, in0=f13[:, :, :, :ch4],
                                            in1=f13[:, :, :, ch4:],
                                            op=mybir.AluOpType.add)
                    logits = spool.tile([P, Gmax * HEADS], F32, tag="logits")
                    nc.vector.tensor_reduce(
                        out=logits[:, :gG * n_h].rearrange(
                            "p (g h) -> p g h", h=n_h),
                        in_=f23,
                        axis=mybir.AxisListType.X, op=mybir.AluOpType.add)
                    # pad-edge mask as additive bias
                    logm = spool.tile([P, Gmax * HEADS], F32, tag="logm")
                    nc.vector.tensor_tensor(
                        out=logm[:, :gG * n_h].rearrange(
                            "p (g h) -> p g h", h=n_h),
                        in0=logits[:, :gG * n_h].rearrange(
                            "p (g h) -> p g h", h=n_h),
                        in1=mb_sb[:, off:off + gG]
                            .rearrange("p (g h) -> p g h", h=1)
                            .to_broadcast([P, gG, n_h]),
                        op=mybir.AluOpType.add)
                    ex = spool.tile([P, Gmax * HEADS], BF16, tag="ex")
                    nc.scalar.activation(ex[:, :gG * n_h], logm[:, :gG * n_h],
                                         mybir.ActivationFunctionType.Exp)
                    # edata = [ex * xl[src] | ex]  (ex broadcast over c_h)
                    edata = epool.tile([P, Gmax * (c_tbl_max + HEADS)], BF16,
                                       tag="edata")
                    ed3 = edata[:, :gG * EC].rearrange("p (g c) -> p g c", c=EC)
                    nc.vector.tensor_tensor(
                        out=ed3[:, :, :C].rearrange(
                            "p g (h c) -> p g h c", c=c_h),
                        in0=xl3[:, :, :C].rearrange(
                            "p g (h c) -> p g h c", c=c_h),
                        in1=ex[:, :gG * n_h].rearrange(
                            "p (g h c) -> p g h c", h=n_h, c=1)
                            .to_broadcast([P, gG, n_h, c_h]),
                        op=mybir.AluOpType.mult)
                    nc.scalar.activation(
                        ed3[:, :, C:],
                        ex[:, :gG * n_h].rearrange("p (g h) -> p g h",
                                                   h=n_h),
                        mybir.ActivationFunctionType.Copy)
                    # segment sums via PE: psum[d, :] += SelT_g^T @ edata_g
                    ps_nd = ppool.tile([P, EC], F32, tag="ps_nd", bufs=2)
                    for g in range(gG):
                        nc.tensor.matmul(
                            ps_nd[:], lhsT=selt[:, g * P:(g + 1) * P],
                            rhs=edata[:, g * EC:(g + 1) * EC],
                            start=(g == 0), stop=(g == gG - 1))
                    den_e = spool.tile([P, HEADS], F32, tag="den_e")
                    nc.vector.tensor_scalar(
                        out=den_e[:, :n_h], in0=ps_nd[:, C:], scalar1=DEN_EPS,
                        scalar2=None, op0=mybir.AluOpType.add)
                    rden = spool.tile([P, HEADS], F32, tag="rden")
                    nc.vector.reciprocal(rden[:, :n_h], den_e[:, :n_h])
                    ob = spool.tile([P, c_tbl_max], F32, tag="ob")
                    nc.vector.tensor_tensor(
                        out=ob[:, :C].rearrange("p (h c) -> p h c", h=n_h),
                        in0=ps_nd[:, :C].rearrange("p (h c) -> p h c", h=n_h),
                        in1=rden[:, :n_h].rearrange("p (h c) -> p h c", c=1)
                            .to_broadcast([P, n_h, c_h]),
                        op=mybir.AluOpType.mult)
                    if l + 1 < nl:
                        hb = hpool.tile([P, c_tbl_max], F16, tag="hb")
                        nc.scalar.activation(hb[:, :C], ob[:, :C],
                                             mybir.ActivationFunctionType.Relu)
                        if dbg and l == 0:
                            nc.sync.dma_start(
                                out=dbg_h1[b * P:(b + 1) * P, :],
                                in_=hb[:, :C])
                            if b == 0:
                                nc.sync.dma_start(out=dbg_lrz[:, :gG * C],
                                                  in_=lrz[:, :gG * C])
                                nc.sync.dma_start(out=dbg_xlg[:, :gG * C],
                                                  in_=xl_g[:, :gG * C])
                                nc.vector.tensor_copy(out=dbg_ex_sb[:, :gG * n_h],
                                                      in_=ex[:, :gG * n_h])
                                nc.sync.dma_start(out=dbg_ex[:, :gG * n_h],
                                                  in_=dbg_ex_sb[:, :gG * n_h])
                        # interleaved next-layer matmul for this block
                        mm_block(l + 1, wl_nxt, wr_nxt, hb[:, :C], b,
                                 do_xl=True, do_xr=True)
                    else:
                        nc.sync.dma_start(
                            out=out_d[b * P:(b + 1) * P, :],
                            in_=ob[:, :cfg.out_real])

                # allgather the next layer's xl table
                if l + 1 < nl:
                    nc.gpsimd.collective_compute(
                        "AllGather", mybir.AluOpType.bypass, replica_groups=rg,
                        ins=[xl_loc[l + 1][:, :].opt()],
                        outs=[xl_tbl[l + 1][:, :].opt()])
                if dbg and l == 0:
                    nc.sync.dma_start(out=dbg_xl0[:, :],
                                      in_=xl_tbl0[:cfg.npc, :])
                    nc.sync.dma_start(out=dbg_xl1[:, :],
                                      in_=xl_loc[1][:, :])
                    nc.sync.dma_start(out=dbg_xr1[:, :],
                                      in_=xr_sb[1][:, :cfg.nblk * 256])
    nc.compile()
    return nc


# ---------------------------------------------------------------------------
# host orchestration
# ---------------------------------------------------------------------------

def _wT_pad(w, c_tbl):
    """w: [h*oc, ic] fp32 -> [ic, c_tbl] fp16 (zero pad the out channels)."""
    w = np.asarray(w, np.float32)
    hoc, ic = w.shape
    out = np.zeros((ic, c_tbl), np.float16)
    out[:, :hoc] = w.T.astype(np.float16)
    return out


def _a_rep(a, c_tbl):
    """a: [h, oc] fp32 -> [128, c_tbl] fp16 replicated across partitions."""
    a = np.asarray(a, np.float32).reshape(-1)
    row = np.zeros(c_tbl, np.float16)
    row[:a.shape[0]] = a.astype(np.float16)
    return np.tile(row[None, :], (P, 1))


def make_in_maps(cfg, G, per_core, x, weights, meta):
    iota = np.tile(np.arange(P, dtype=np.float16)[None, :], (P, 1))
    iotac = np.tile(np.arange(P, dtype=np.float16)[:, None], (1, max(G) * P))
    x16 = np.asarray(x, np.float32).astype(np.float16)
    xpad = np.zeros((cfg.npad, cfg.layers[0]["c_in"]), np.float16)
    real = meta["perm"] >= 0
    xpad[real] = x16[meta["perm"][real]]
    xpadT = np.ascontiguousarray(xpad.T)
    shared = dict(iota=iota, iotac=iotac, x_fullT=xpadT)
    for l, L in enumerate(cfg.layers):
        wl, wr, a = weights[l]
        shared[f"w{l}l"] = _wT_pad(wl, L["c_tbl"])
        shared[f"w{l}r"] = _wT_pad(wr, L["c_cmp"])
        shared[f"a{l}"] = _a_rep(a, L["c_cmp"])
    in_maps = []
    for c in range(cfg.n_cores):
        m = dict(shared)
        m["x_ownT"] = np.ascontiguousarray(
            xpadT[:, c * cfg.npc:(c + 1) * cfg.npc])
        m.update(per_core[c])
        in_maps.append(m)
    return in_maps


_CACHE = {}


def _get_built(cfg, edge_index):
    key = hash(np.asarray(edge_index).tobytes())
    if key not in _CACHE:
        G, per_core, meta = prep_graph(cfg, edge_index)
        nc = build_nc(cfg, G)
        _CACHE[key] = (G, per_core, meta, nc)
    return _CACHE[key]


def kernel(x, edge_index,
           w1l, b1l, w1r, b1r, a1, bo1,
           w2l, b2l, w2r, b2r, a2, bo2,
           w3l, b3l, w3r, b3r, a3, bo3,
           w4l, b4l, w4r, b4r, a4, bo4,
           _trace=False):
    cfg = real_cfg()
    for b in (b1l, b1r, b2l, b2r, b3l, b3r, b4l, b4r, bo1, bo2, bo3):
        assert np.max(np.abs(np.asarray(b, np.float32))) == 0.0, \
            "non-zero internal biases not supported"
    G, per_core, meta, nc = _get_built(cfg, edge_index)
    weights = [(w1l, w1r, a1), (w2l, w2r, a2), (w3l, w3r, a3), (w4l, w4r, a4)]
    in_maps = make_in_maps(cfg, G, per_core, x, weights, meta)
    res = run_bass_kernel_spmd(nc, in_maps, core_ids=list(range(cfg.n_cores)),
                               trace=_trace)
    outs = [np.asarray(res.results[c]["out"]) for c in range(cfg.n_cores)]
    full_pos = np.concatenate(outs, axis=0).astype(np.float32)
    full = full_pos[meta["ipos"]]
    full = full + np.asarray(bo4, np.float32)[None, :]
    if _trace:
        kernel.last_exec_time_ns = res.exec_time_ns
        kernel.last_res = res
    return full


kernel.last_exec_time_ns = None
kernel.last_res = None


# revision 30
# speedup vs baseline: 1.2605x; 1.0103x over previous
"""4-layer GATv2 forward pass on 8 TRN2 NeuronCores (Bass/Tile).

Strategy (node/dst partitioning, no cross-core segment reductions):
  - Nodes are padded to 20480 and split into 8 contiguous slices of 2560
    (20 blocks of 128 dst nodes per core).  Each core owns the segment
    softmax + weighted scatter for its dst nodes, so all softmax
    reductions are core-local.
  - Edges (with self loops appended) are routed to the core/block that
    owns their dst.  Per (core, block) edge counts are padded to a
    shared multiple of 128 (G[b] groups of 128 edges) so one NEFF works
    for all 8 cores.
  - Layer 1 source transforms (xl) are computed for the FULL node table
    on every core (x is replicated), so no collective is needed before
    the first edge phase.  For layers 2-4, the matmul for layer l+1 of
    dst block b is interleaved right after the edge phase of block b
    (its input h lives in SBUF), so the AllGather of the next xl table
    can issue the moment the edge phase drains.
  - Per-edge work is edge-major (partition = edge % 128): dma_gather of
    xl[src] rows (parallel calls over the 4 SWDGE queues), xr[dst]
    expansion + z = xl+xr + leaky-relu via PE matmuls against a one-hot
    dst selector, per-head dot with `a` (DVE folds+reduce), exp, then
    the softmax denominator and alpha-weighted sum of xl[src] as one PE
    matmul per 128-edge group (SelT).
  - Softmax uses exp(logit) directly (no running max): logits are O(10)
    here, fp32 exp is exact enough, and the math is identical to the
    reference's shifted softmax.

kernel(**inputs) takes the full problem inputs and returns the full
[20000, 16] fp32 output.
"""

import numpy as np

import concourse.bass as bass
import concourse.bacc as bacc
import concourse.mybir as mybir
import concourse.tile as tile
from concourse.bass_utils import run_bass_kernel_spmd
from concourse.masks import make_identity

F16 = mybir.dt.float16
BF16 = mybir.dt.bfloat16
F32 = mybir.dt.float32
I16 = mybir.dt.int16
U8 = mybir.dt.uint8
P = 128

# model dims (fixed by the problem)
N_REAL = 20000
E_RAW = 320000
IN_CH = 128
HID = 64
HEADS = 4
OUT_CH = 16
SLOPE = 0.2

MASK_NEG = -50.0  # additive logit bias for pad edges
DEN_EPS = 1e-12   # keeps reciprocal() in range for edgeless (pad) dst rows


class Cfg:
    def __init__(self, n_cores, npc, n_real, layers, out_real):
        assert npc % P == 0
        self.n_cores = n_cores
        self.npc = npc              # nodes per core (padded)
        self.nblk = npc // P        # dst blocks per core
        self.n_real = n_real
        self.npad = n_cores * npc
        self.layers = layers        # list of dicts: c_in, c_tbl, n_h, c_h
        self.out_real = out_real    # real output channels of last layer


def real_cfg():
    layers = [
        dict(c_in=IN_CH, c_tbl=HEADS * HID, c_cmp=HEADS * HID,
             n_h=HEADS, c_h=HID),
        dict(c_in=HEADS * HID, c_tbl=HEADS * HID, c_cmp=HEADS * HID,
             n_h=HEADS, c_h=HID),
        dict(c_in=HEADS * HID, c_tbl=HEADS * HID, c_cmp=HEADS * HID,
             n_h=HEADS, c_h=HID),
        # 16 real out channels: gather table stays 128 wide (256B descriptor
        # minimum) but all edge-phase compute runs at width 32
        dict(c_in=HEADS * HID, c_tbl=P, c_cmp=32, n_h=1, c_h=32),
    ]
    return Cfg(8, 2560, N_REAL, layers, OUT_CH)


# ---------------------------------------------------------------------------
# host-side graph preprocessing
# ---------------------------------------------------------------------------

def prep_graph(cfg, edge_index):
    """Route edges (plus self loops) to (core, block) by dst; build per-core
    gather-index / dst-local / mask arrays in the exact SBUF layouts the
    kernel consumes.

    Dst nodes are REASSIGNED to (core, block) bins by LPT load balancing
    (highest in-degree first, always into the lightest non-full bin) so
    per-bin edge counts are near-uniform: the shared padded group count
    drops from ceil(max/128) to ceil(mean/128).  All device-side arrays
    are expressed in permuted "position" space; kernel() un-permutes the
    output rows at the end (meta["ipos"])."""
    n = cfg.n_real
    src = np.concatenate([np.asarray(edge_index[0], np.int64),
                          np.arange(n, dtype=np.int64)])
    dst = np.concatenate([np.asarray(edge_index[1], np.int64),
                          np.arange(n, dtype=np.int64)])
    assert src.min() >= 0 and src.max() < n and dst.min() >= 0 and dst.max() < n

    nbin = cfg.n_cores * cfg.nblk
    deg = np.bincount(dst, minlength=n)
    nodes_by_deg = np.argsort(-deg, kind="stable")
    binsum = np.zeros(nbin, np.int64)
    bincnt = np.zeros(nbin, np.int64)
    ipos = np.empty(n, np.int64)
    perm = np.full(cfg.npad, -1, np.int64)
    for v in nodes_by_deg:
        open_b = np.flatnonzero(bincnt < P)
        b = open_b[np.argmin(binsum[open_b])]
        p_ = b * P + bincnt[b]
        ipos[v] = p_
        perm[p_] = v
        binsum[b] += deg[v]
        bincnt[b] += 1
    meta = dict(perm=perm, ipos=ipos)

    src, dst = ipos[src], ipos[dst]       # positions from here on
    gblk = dst // P                       # global block id (core-major)
    order = np.argsort(gblk, kind="stable")
    src, dst, gblk = src[order], dst[order], gblk[order]

    nblk_tot = cfg.n_cores * cfg.nblk
    counts = np.bincount(gblk, minlength=nblk_tot).reshape(cfg.n_cores, cfg.nblk)
    G = np.maximum(1, (counts.max(axis=0) + P - 1) // P).astype(np.int64)  # [nblk]
    W = int(G.sum())

    # split edges per (core, block)
    starts = np.zeros(nblk_tot + 1, np.int64)
    np.cumsum(counts.reshape(-1), out=starts[1:])

    per_core = []
    for c in range(cfg.n_cores):
        xl_idx = np.zeros((P, 8 * W), np.int16)
        dloc = np.zeros((P, W), np.float16)
        dlocT = np.zeros((1, W * P), np.float16)
        mbias = np.full((P, W), MASK_NEG, np.float16)
        off = 0
        for b in range(cfg.nblk):
            gb = c * cfg.nblk + b
            s, e = starts[gb], starts[gb + 1]
            nreal = int(e - s)
            npad_e = int(G[b]) * P
            fsrc = np.zeros(npad_e, np.int64)
            fdl = np.zeros(npad_e, np.int64)
            fm = np.full(npad_e, MASK_NEG, np.float32)
            fsrc[:nreal] = src[s:e]
            fdl[:nreal] = dst[s:e] % P
            fm[:nreal] = 0.0
            # edge i -> partition i % 128, group i // 128
            dloc[:, off:off + G[b]] = fdl.reshape(G[b], P).T.astype(np.float16)
            dlocT[0, off * P:(off + int(G[b])) * P] = fdl.astype(np.float16)
            mbias[:, off:off + G[b]] = fm.reshape(G[b], P).T.astype(np.float16)
            # wrapped idx layout: wrapped[p, s] = flat[s*16 + p], replicated
            # into all 8 16-partition groups (one per GPSIMD Q7 core)
            xl_idx[:, 8 * off:8 * (off + G[b])] = np.tile(
                fsrc.astype(np.int16).reshape(-1, 16).T, (8, 1))
            off += int(G[b])
        per_core.append(dict(xl_idx=xl_idx, dloc=dloc,
                             dlocT=np.tile(dlocT, (P, 1)), mbias=mbias))
    return [int(g) for g in G], per_core, meta


# ---------------------------------------------------------------------------
# bass program
# ---------------------------------------------------------------------------

def build_nc(cfg, G):
    """Build the (single, SPMD) bass program."""
    nl = len(cfg.layers)
    W = sum(G)
    Gmax = max(G)
    c_tbl_max = max(L["c_tbl"] for L in cfg.layers)
    kc_max = max(L["c_in"] for L in cfg.layers) // P
    nblk_tbl = cfg.npad // P            # full-table blocks (layer-1 xl)

    nc = bacc.Bacc("TRN2", target_bir_lowering=False, debug=False,
                   num_devices=cfg.n_cores, num_swdge_queues=4)

    # layer-1 input, pre-transposed on the host: [c_in, nodes]
    x_fullT = nc.dram_tensor("x_fullT", [cfg.layers[0]["c_in"], cfg.npad], F16,
                             kind="ExternalInput")
    x_ownT = nc.dram_tensor("x_ownT", [cfg.layers[0]["c_in"], cfg.npc], F16,
                            kind="ExternalInput")
    xl_idx_d = nc.dram_tensor("xl_idx", [P, 8 * W], I16, kind="ExternalInput")
    dloc_d = nc.dram_tensor("dloc", [P, W], F16, kind="ExternalInput")
    dlocT_d = nc.dram_tensor("dlocT", [P, W * P], F16, kind="ExternalInput")
    iotac_d = nc.dram_tensor("iotac", [P, Gmax * P], F16, kind="ExternalInput")
    mbias_d = nc.dram_tensor("mbias", [P, W], F16, kind="ExternalInput")
    iota_d = nc.dram_tensor("iota", [P, P], F16, kind="ExternalInput")
    w_d, a_d = [], []
    for l, L in enumerate(cfg.layers):
        wl = nc.dram_tensor(f"w{l}l", [L["c_in"], L["c_tbl"]], F16,
                            kind="ExternalInput")
        wr = nc.dram_tensor(f"w{l}r", [L["c_in"], L["c_cmp"]], F16,
                            kind="ExternalInput")
        w_d.append((wl, wr))
        a_d.append(nc.dram_tensor(f"a{l}", [P, L["c_cmp"]], F16,
                                  kind="ExternalInput"))
    out_d = nc.dram_tensor("out", [cfg.npc, cfg.out_real], F32,
                           kind="ExternalOutput")
    import os
    dbg = os.environ.get("K_DEBUG") == "1"
    if dbg:
        dbg_xl0 = nc.dram_tensor("dbg_xl0", [cfg.npc, cfg.layers[0]["c_tbl"]],
                                 F16, kind="ExternalOutput")
        dbg_xl1 = nc.dram_tensor("dbg_xl1", [cfg.npc, cfg.layers[1]["c_tbl"]],
                                 F16, kind="ExternalOutput")
        dbg_xr1 = nc.dram_tensor("dbg_xr1", [P, cfg.nblk * 256], F16,
                                 kind="ExternalOutput")
        dbg_h1 = nc.dram_tensor("dbg_h1", [cfg.npc, 256], F16,
                                 kind="ExternalOutput")
        dbg_lrz = nc.dram_tensor("dbg_lrz", [P, 18 * 256], F16,
                                 kind="ExternalOutput")
        dbg_xlg = nc.dram_tensor("dbg_xlg", [P, 18 * 256], F16,
                                 kind="ExternalOutput")
        dbg_ex = nc.dram_tensor("dbg_ex", [P, 18 * HEADS], F16,
                                kind="ExternalOutput")

    rg = [list(range(cfg.n_cores))]

    with tile.TileContext(nc) as tc:
        with (
            tc.tile_pool(name="const", bufs=1) as cpool,
            tc.tile_pool(name="wts", bufs=2) as wpool,
            tc.tile_pool(name="mm", bufs=3) as mpool,
            tc.tile_pool(name="gath", bufs=5) as gpool,
            tc.tile_pool(name="gidx", bufs=20) as gipool,
            tc.tile_pool(name="edge", bufs=2) as epool,
            tc.tile_pool(name="small", bufs=2) as spool,
            tc.tile_pool(name="hbuf", bufs=4) as hpool,
            tc.tile_pool(name="xrsb", bufs=2) as xpool,
            tc.tile_pool(name="psum", bufs=2, space="PSUM") as ppool,
            tc.tile_pool(name="dram", bufs=1, space="DRAM") as dpool,
        ):
            # ---- persistent constants -------------------------------------
            iota_sb = cpool.tile([P, P], F16, tag="iota")
            nc.sync.dma_start(out=iota_sb[:], in_=iota_d[:])
            ident = cpool.tile([P, P], F16, tag="ident")
            make_identity(nc, ident[:])
            dloc_sb = cpool.tile([P, W], F16, tag="dloc")
            nc.sync.dma_start(out=dloc_sb[:], in_=dloc_d[:])
            mb_sb = cpool.tile([P, W], F16, tag="mbias")
            nc.sync.dma_start(out=mb_sb[:], in_=mbias_d[:])
            # iotac_u8[p, e] = p  (host-replicated)
            iotac_sb = cpool.tile([P, Gmax * P], F16, tag="iotacr")
            nc.sync.dma_start(out=iotac_sb[:], in_=iotac_d[:])
            if dbg:
                dbg_ex_sb = cpool.tile([P, 18 * HEADS], F16, tag="dbgex")

            # ---- per-layer DRAM scratch -----------------------------------
            xl_loc, xl_tbl = [None], [None]
            # layer 0 xl table is computed fully on every core -> Local.
            xl_tbl0 = dpool.tile([cfg.npad, cfg.layers[0]["c_tbl"]], F16,
                                 tag="xltbl0", name="xltbl0")
            for l in range(1, nl):
                L = cfg.layers[l]
                xl_loc.append(dpool.tile([cfg.npc, L["c_tbl"]], F16,
                                         tag=f"xlloc{l}", name=f"xlloc{l}"))
                xl_tbl.append(dpool.tile(
                    [cfg.npad, L["c_tbl"]], F16, tag=f"xltbl{l}",
                    name=f"xltbl{l}", addr_space="Shared"))
            xl_tbl[0] = xl_tbl0

            # per-layer xr tables stay in SBUF (written by the interleaved
            # matmul of the previous layer's edge phase)
            xr_sb = [xpool.tile([P, cfg.nblk * c_tbl_max], F16, tag="xr_sb",
                                name=f"xr_sb{l}")
                     for l in range(nl)]

            # weight tiles per layer (wpool rotates 2 buffers)
            def load_weights(l):
                L = cfg.layers[l]
                Cl, Cr, c_in = L["c_tbl"], L["c_cmp"], L["c_in"]
                kc_n = c_in // P
                wl_sb = wpool.tile([P, kc_max * c_tbl_max], F16, tag="wl")
                wr_sb = wpool.tile([P, kc_max * c_tbl_max], F16, tag="wr")
                for kc in range(kc_n):
                    nc.sync.dma_start(out=wl_sb[:, kc * Cl:(kc + 1) * Cl],
                                      in_=w_d[l][0][kc * P:(kc + 1) * P, :])
                    nc.sync.dma_start(out=wr_sb[:, kc * Cr:(kc + 1) * Cr],
                                      in_=w_d[l][1][kc * P:(kc + 1) * P, :])
                return wl_sb, wr_sb

            def load_arep(l):
                L = cfg.layers[l]
                C = L["c_cmp"]
                a_rep = wpool.tile([P, Gmax * c_tbl_max], F16, tag="arep")
                nc.sync.dma_start(
                    out=a_rep[:, :Gmax * C].rearrange("p (g c) -> p g c", g=Gmax),
                    in_=a_d[l][:].rearrange("p (g c) -> p g c", g=1)
                        .to_broadcast([P, Gmax, C]))
                return a_rep

            def mm_block(l, wl_sb, wr_sb, h_ap, blk, do_xl, do_xr):
                """Source/target transforms of one 128-node block of layer l.
                h_ap: [P, c_in] SBUF activation tile; writes xl to
                xl_loc/xl_tbl and/or xr into the resident xr_sb table."""
                L = cfg.layers[l]
                Cl, Cr, c_in = L["c_tbl"], L["c_cmp"], L["c_in"]
                kc_n = c_in // P
                hT = mpool.tile([P, kc_max * P], F16, tag="hT")
                for kc in range(kc_n):
                    pt = ppool.tile([P, P], F16, tag="pt", bufs=1)
                    nc.tensor.transpose(pt[:], h_ap[:, kc * P:(kc + 1) * P],
                                        ident[:])
                    nc.vector.tensor_copy(out=hT[:, kc * P:(kc + 1) * P],
                                          in_=pt[:])
                if do_xl:
                    ps_xl = ppool.tile([P, c_tbl_max], F32, tag="ps_mm")
                    for kc in range(kc_n):
                        nc.tensor.matmul(ps_xl[:, :Cl],
                                         lhsT=hT[:, kc * P:(kc + 1) * P],
                                         rhs=wl_sb[:, kc * Cl:(kc + 1) * Cl],
                                         start=(kc == 0), stop=(kc == kc_n - 1))
                    xl_t = mpool.tile([P, c_tbl_max], F16, tag="xl_t")
                    nc.scalar.activation(xl_t[:, :Cl], ps_xl[:, :Cl],
                                         mybir.ActivationFunctionType.Copy)
                    if l == 0:
                        nc.sync.dma_start(
                            out=xl_tbl0[blk * P:(blk + 1) * P, :],
                            in_=xl_t[:, :Cl])
                    else:
                        nc.sync.dma_start(
                            out=xl_loc[l][blk * P:(blk + 1) * P, :],
                            in_=xl_t[:, :Cl])
                if do_xr:
                    ps_xr = ppool.tile([P, c_tbl_max], F32, tag="ps_mm")
                    for kc in range(kc_n):
                        nc.tensor.matmul(ps_xr[:, :Cr],
                                         lhsT=hT[:, kc * P:(kc + 1) * P],
                                         rhs=wr_sb[:, kc * Cr:(kc + 1) * Cr],
                                         start=(kc == 0), stop=(kc == kc_n - 1))
                    nc.scalar.activation(
                        xr_sb[l][:, blk * c_tbl_max:blk * c_tbl_max + Cr],
                        ps_xr[:, :Cr], mybir.ActivationFunctionType.Copy)

            # ---- layer 0 matmul phase: full xl table, local xr ------------
            # x arrives pre-transposed, so each block's lhsT is a direct
            # slice load: no PE transposes, batched 4-block DMAs.
            wl_sb, wr_sb = load_weights(0)
            c_in0 = cfg.layers[0]["c_in"]
            C0 = cfg.layers[0]["c_tbl"]
            TB = 4
            for t0 in range(0, nblk_tbl, TB):
                xT4 = mpool.tile([P, TB * P], F16, tag="xT4")
                nc.sync.dma_start(out=xT4[:],
                                  in_=x_fullT[:, t0 * P:(t0 + TB) * P])
                xl_t4 = mpool.tile([P, TB * C0], F16, tag="xl_t4")
                for ti in range(TB):
                    ps_xl = ppool.tile([P, c_tbl_max], F32, tag="ps_mm")
                    nc.tensor.matmul(ps_xl[:, :C0],
                                     lhsT=xT4[:, ti * P:(ti + 1) * P],
                                     rhs=wl_sb[:, :C0], start=True, stop=True)
                    if ti % 2 == 0:
                        nc.scalar.activation(xl_t4[:, ti * C0:(ti + 1) * C0],
                                             ps_xl[:, :C0],
                                             mybir.ActivationFunctionType.Copy)
                    else:
                        nc.vector.tensor_copy(
                            out=xl_t4[:, ti * C0:(ti + 1) * C0],
                            in_=ps_xl[:, :C0])
                nc.sync.dma_start(
                    out=xl_tbl0[t0 * P:(t0 + TB) * P, :]
                        .rearrange("(t p) c -> p t c", p=P),
                    in_=xl_t4[:].rearrange("p (t c) -> p t c", t=TB))
            for b0 in range(0, cfg.nblk, TB):
                xT4 = mpool.tile([P, TB * P], F16, tag="xT4")
                nc.sync.dma_start(out=xT4[:],
                                  in_=x_ownT[:, b0 * P:(b0 + TB) * P])
                for bi in range(TB):
                    b = b0 + bi
                    ps_xr = ppool.tile([P, c_tbl_max], F32, tag="ps_mm")
                    nc.tensor.matmul(ps_xr[:, :C0],
                                     lhsT=xT4[:, bi * P:(bi + 1) * P],
                                     rhs=wr_sb[:, :C0], start=True, stop=True)
                    nc.scalar.activation(
                        xr_sb[0][:, b * c_tbl_max:b * c_tbl_max + C0],
                        ps_xr[:, :C0], mybir.ActivationFunctionType.Copy)

            # ---- per-layer edge phase (+ interleaved next-layer matmul) ---
            qn = [0]

            def gather_rows(tbl_ap, out_tile, off_g, n_g, C_, nm):
                """Gather n_g*128 rows from tbl_ap into out_tile
                [P, n_g, C_], split into <=6-group chunks spread over the
                4 SWDGE queues (no ordering chain: they run concurrently)."""
                CH = 5
                for k0 in range(0, n_g, CH):
                    gk = min(CH, n_g - k0)
                    it = gipool.tile([P, 8 * CH], I16, tag="idxt",
                                     name=f"idxt_{nm}_{k0}")
                    nc.sync.dma_start(
                        out=it[:, :8 * gk],
                        in_=xl_idx_d[:, 8 * (off_g + k0):
                                     8 * (off_g + k0 + gk)])
                    nc.gpsimd.dma_gather(
                        out_ap=out_tile[:, k0 * C_:(k0 + gk) * C_]
                            .rearrange("p (g c) -> p g c", c=C_),
                        in_ap=tbl_ap,
                        idxs_ap=it[:, :8 * gk],
                        num_idxs=gk * P, num_idxs_reg=gk * P,
                        elem_size=C_, queue_num=qn[0] % 4)
                    qn[0] += 1

            for l, L in enumerate(cfg.layers):
                c_in, C, n_h, c_h = L["c_in"], L["c_cmp"], L["n_h"], L["c_h"]
                CT = L["c_tbl"]
                EC = C + n_h
                a_rep = load_arep(l)
                if l + 1 < nl:
                    wl_nxt, wr_nxt = load_weights(l + 1)

                for b in range(cfg.nblk):
                    gG = G[b]
                    off = sum(G[:b])
                    xl_g = gpool.tile([P, Gmax * c_tbl_max], F16, tag="xl_g")
                    gather_rows(xl_tbl[l][:, :], xl_g, off, gG, CT,
                                f"xl{l}_{b}")
                    xl3 = xl_g[:, :gG * CT].rearrange(
                        "p (g c) -> p g c", c=CT)
                    xr_blk = xr_sb[l][:, b * c_tbl_max:b * c_tbl_max + C]
                    # Sel[d, e] = (d == dloc[e])    [for xr expansion]
                    dlt = epool.tile([P, Gmax * P], F16, tag="dlt")
                    nc.sync.dma_start(out=dlt[:, :gG * P],
                                      in_=dlocT_d[:, off * P:(off + gG) * P])
                    sel = epool.tile([P, Gmax * P], F16, tag="sel")
                    nc.vector.tensor_tensor(
                        out=sel[:, :gG * P], in0=dlt[:, :gG * P],
                        in1=iotac_sb[:, :gG * P],
                        op=mybir.AluOpType.is_equal)
                    # SelT[e, d] = (dloc[e] == d)   [for num/den matmuls]
                    selt = epool.tile([P, Gmax * P], BF16, tag="selt")
                    nc.vector.tensor_tensor(
                        out=selt[:, :gG * P].rearrange(
                            "p (g d) -> p g d", d=P),
                        in0=dloc_sb[:, off:off + gG]
                            .rearrange("p (g d) -> p g d", d=1)
                            .to_broadcast([P, gG, P]),
                        in1=iota_sb[:].rearrange("p (g d) -> p g d", g=1)
                            .to_broadcast([P, gG, P]),
                        op=mybir.AluOpType.is_equal)
                    # z (per quad of groups) in PSUM:
                    #   z_g = Sel_g^T @ xr_blk + I^T @ xl_g   -> leaky relu
                    lrz = epool.tile([P, Gmax * c_tbl_max], F16, tag="lrz")
                    for g0 in range(0, gG, 2):
                        gns = min(2, gG - g0)
                        ps_z = ppool.tile([P, 2 * c_tbl_max], F32, tag="ps_z",
                                          bufs=3)
                        # NOTE: each slice's start->stop matmul pair must stay
                        # tightly sequential; interleaving several open
                        # accumulation groups corrupts PSUM on this HW.
                        for gg in range(g0, g0 + gns):
                            sl = slice((gg - g0) * C, (gg - g0 + 1) * C)
                            nc.tensor.matmul(
                                ps_z[:, sl], lhsT=sel[:, gg * P:(gg + 1) * P],
                                rhs=xr_blk, start=True, stop=False)
                            nc.tensor.matmul(
                                ps_z[:, sl], lhsT=ident[:],
                                rhs=xl_g[:, gg * CT:gg * CT + C],
                                start=False, stop=True)
                        nc.scalar.activation(
                            lrz[:, g0 * C:(g0 + gns) * C],
                            ps_z[:, :gns * C],
                            mybir.ActivationFunctionType.Prelu,
                            alpha=SLOPE)
                    # a * LR(z)
                    alr = epool.tile([P, Gmax * c_tbl_max], F16, tag="alr")
                    nc.vector.tensor_tensor(out=alr[:, :gG * C],
                                            in0=lrz[:, :gG * C],
                                            in1=a_rep[:, :gG * C],
                                            op=mybir.AluOpType.mult)
                    # logits: two folds + reduce over c_h/4
                    ch2, ch4 = c_h // 2, c_h // 4
                    fold1 = spool.tile([P, Gmax * c_tbl_max // 2], F16,
                                       tag="fold1")
                    a4 = alr[:, :gG * C].rearrange(
                        "p (g h c) -> p g h c", h=n_h, c=c_h)
                    f13 = fold1[:, :gG * C // 2].rearrange(
                        "p (g h c) -> p g h c", h=n_h, c=ch2)
                    nc.vector.tensor_tensor(out=f13, in0=a4[:, :, :, :ch2],
                                            in1=a4[:, :, :, ch2:],
                                            op=mybir.AluOpType.add)
                    fold2 = spool.tile([P, Gmax * c_tbl_max // 4], F16,
                                       tag="fold2")
                    f23 = fold2[:, :gG * C // 4].rearrange(
                        "p (g h c) -> p g h c", h=n_h, c=ch4)
                    nc.vector.tensor_tensor(out=f23, in0=f13[:, :, :, :ch4],
                                            in1=f13[:, :, :, ch4:],
                                            op=mybir.AluOpType.add)
                    logits = spool.tile([P, Gmax * HEADS], F32, tag="logits")
                    nc.vector.tensor_reduce(
                        out=logits[:, :gG * n_h].rearrange(
                            "p (g h) -> p g h", h=n_h),
                        in_=f23,
                        axis=mybir.AxisListType.X, op=mybir.AluOpType.add)
                    # pad-edge mask as additive bias
                    logm = spool.tile([P, Gmax * HEADS], F32, tag="logm")
                    nc.vector.tensor_tensor(
                        out=logm[:, :gG * n_h].rearrange(
                            "p (g h) -> p g h", h=n_h),
                        in0=logits[:, :gG * n_h].rearrange(
                            "p (g h) -> p g h", h=n_h),
                        in1=mb_sb[:, off:off + gG]
                            .rearrange("p (g h) -> p g h", h=1)
                            .to_broadcast([P, gG, n_h]),
                        op=mybir.AluOpType.add)
                    ex = spool.tile([P, Gmax * HEADS], BF16, tag="ex")
                    nc.scalar.activation(ex[:, :gG * n_h], logm[:, :gG * n_h],
                                         mybir.ActivationFunctionType.Exp)
                    # edata = [ex * xl[src] | ex]  (ex broadcast over c_h)
                    edata = epool.tile([P, Gmax * (c_tbl_max + HEADS)], BF16,
                                       tag="edata")
                    ed3 = edata[:, :gG * EC].rearrange("p (g c) -> p g c", c=EC)
                    nc.vector.tensor_tensor(
                        out=ed3[:, :, :C].rearrange(
                            "p g (h c) -> p g h c", c=c_h),
                        in0=xl3[:, :, :C].rearrange(
                            "p g (h c) -> p g h c", c=c_h),
                        in1=ex[:, :gG * n_h].rearrange(
                            "p (g h c) -> p g h c", h=n_h, c=1)
                            .to_broadcast([P, gG, n_h, c_h]),
                        op=mybir.AluOpType.mult)
                    nc.scalar.activation(
                        ed3[:, :, C:],
                        ex[:, :gG * n_h].rearrange("p (g h) -> p g h",
                                                   h=n_h),
                        mybir.ActivationFunctionType.Copy)
                    # segment sums via PE: psum[d, :] += SelT_g^T @ edata_g
                    ps_nd = ppool.tile([P, EC], F32, tag="ps_nd", bufs=2)
                    for g in range(gG):
                        nc.tensor.matmul(
                            ps_nd[:], lhsT=selt[:, g * P:(g + 1) * P],
                            rhs=edata[:, g * EC:(g + 1) * EC],
                            start=(g == 0), stop=(g == gG - 1))
                    den_e = spool.tile([P, HEADS], F32, tag="den_e")
                    nc.vector.tensor_scalar(
                        out=den_e[:, :n_h], in0=ps_nd[:, C:], scalar1=DEN_EPS,
                        scalar2=None, op0=mybir.AluOpType.add)
                    rden = spool.tile([P, HEADS], F32, tag="rden")
                    nc.vector.reciprocal(rden[:, :n_h], den_e[:, :n_h])
                    ob = spool.tile([P, c_tbl_max], F32, tag="ob")
                    nc.vector.tensor_tensor(
                        out=ob[:, :C].rearrange("p (h c) -> p h c", h=n_h),
                        in0=ps_nd[:, :C].rearrange("p (h c) -> p h c", h=n_h),
                        in1=rden[:, :n_h].rearrange("p (h c) -> p h c", c=1)
                            .to_broadcast([P, n_h, c_h]),
                        op=mybir.AluOpType.mult)
                    if l + 1 < nl:
                        hb = hpool.tile([P, c_tbl_max], F16, tag="hb")
                        nc.scalar.activation(hb[:, :C], ob[:, :C],
                                             mybir.ActivationFunctionType.Relu)
                        if dbg and l == 0:
                            nc.sync.dma_start(
                                out=dbg_h1[b * P:(b + 1) * P, :],
                                in_=hb[:, :C])
                            if b == 0:
                                nc.sync.dma_start(out=dbg_lrz[:, :gG * C],
                                                  in_=lrz[:, :gG * C])
                                nc.sync.dma_start(out=dbg_xlg[:, :gG * C],
                                                  in_=xl_g[:, :gG * C])
                                nc.vector.tensor_copy(out=dbg_ex_sb[:, :gG * n_h],
                                                      in_=ex[:, :gG * n_h])
                                nc.sync.dma_start(out=dbg_ex[:, :gG * n_h],
                                                  in_=dbg_ex_sb[:, :gG * n_h])
                        # interleaved next-layer matmul for this block
                        mm_block(l + 1, wl_nxt, wr_nxt, hb[:, :C], b,
                                 do_xl=True, do_xr=True)
                    else:
                        nc.sync.dma_start(
                            out=out_d[b * P:(b + 1) * P, :],
                            in_=ob[:, :cfg.out_real])

                # allgather the next layer's xl table
                if l + 1 < nl:
                    nc.gpsimd.collective_compute(
                        "AllGather", mybir.AluOpType.bypass, replica_groups=rg,
                        ins=[xl_loc[l + 1][:, :].opt()],
                        outs=[xl_tbl[l + 1][:, :].opt()])
                if dbg and l == 0:
                    nc.sync.dma_start(out=dbg_xl0[:, :],
                                      in_=xl_tbl0[:cfg.npc, :])
                    nc.sync.dma_start(out=dbg_xl1[:, :],
                                      in_=xl_loc[1][:, :])
                    nc.sync.dma_start(out=dbg_xr1[:, :],
                                      in_=xr_sb[1][:, :cfg.nblk * 256])
    nc.compile()
    return nc


# ---------------------------------------------------------------------------
# host orchestration
# ---------------------------------------------------------------------------

def _wT_pad(w, c_tbl):
    """w: [h*oc, ic] fp32 -> [ic, c_tbl] fp16 (zero pad the out channels)."""
    w = np.asarray(w, np.float32)
    hoc, ic = w.shape
    out = np.zeros((ic, c_tbl), np.float16)
    out[:, :hoc] = w.T.astype(np.float16)
    return out


def _a_rep(a, c_tbl):
    """a: [h, oc] fp32 -> [128, c_tbl] fp16 replicated across partitions."""
    a = np.asarray(a, np.float32).reshape(-1)
    row = np.zeros(c_tbl, np.float16)
    row[:a.shape[0]] = a.astype(np.float16)
    return np.tile(row[None, :], (P, 1))


def make_in_maps(cfg, G, per_core, x, weights, meta):
    iota = np.tile(np.arange(P, dtype=np.float16)[None, :], (P, 1))
    iotac = np.tile(np.arange(P, dtype=np.float16)[:, None], (1, max(G) * P))
    x16 = np.asarray(x, np.float32).astype(np.float16)
    xpad = np.zeros((cfg.npad, cfg.layers[0]["c_in"]), np.float16)
    real = meta["perm"] >= 0
    xpad[real] = x16[meta["perm"][real]]
    xpadT = np.ascontiguousarray(xpad.T)
    shared = dict(iota=iota, iotac=iotac, x_fullT=xpadT)
    for l, L in enumerate(cfg.layers):
        wl, wr, a = weights[l]
        shared[f"w{l}l"] = _wT_pad(wl, L["c_tbl"])
        shared[f"w{l}r"] = _wT_pad(wr, L["c_cmp"])
        shared[f"a{l}"] = _a_rep(a, L["c_cmp"])
    in_maps = []
    for c in range(cfg.n_cores):
        m = dict(shared)
        m["x_ownT"] = np.ascontiguousarray(
            xpadT[:, c * cfg.npc:(c + 1) * cfg.npc])
        m.update(per_core[c])
        in_maps.append(m)
    return in_maps


_CACHE = {}


def _get_built(cfg, edge_index):
    key = hash(np.asarray(edge_index).tobytes())
    if key not in _CACHE:
        G, per_core, meta = prep_graph(cfg, edge_index)
        nc = build_nc(cfg, G)
        _CACHE[key] = (G, per_core, meta, nc)
    return _CACHE[key]


def kernel(x, edge_index,
           w1l, b1l, w1r, b1r, a1, bo1,
           w2l, b2l, w2r, b2r, a2, bo2,
           w3l, b3l, w3r, b3r, a3, bo3,
           w4l, b4l, w4r, b4r, a4, bo4,
           _trace=False):
    cfg = real_cfg()
    for b in (b1l, b1r, b2l, b2r, b3l, b3r, b4l, b4r, bo1, bo2, bo3):
        assert np.max(np.abs(np.asarray(b, np.float32))) == 0.0, \
            "non-zero internal biases not supported"
    G, per_core, meta, nc = _get_built(cfg, edge_index)
    weights = [(w1l, w1r, a1), (w2l, w2r, a2), (w3l, w3r, a3), (w4l, w4r, a4)]
    in_maps = make_in_maps(cfg, G, per_core, x, weights, meta)
    res = run_bass_kernel_spmd(nc, in_maps, core_ids=list(range(cfg.n_cores)),
                               trace=_trace)
    outs = [np.asarray(res.results[c]["out"]) for c in range(cfg.n_cores)]
    full_pos = np.concatenate(outs, axis=0).astype(np.float32)
    full = full_pos[meta["ipos"]]
    full = full + np.asarray(bo4, np.float32)[None, :]
    if _trace:
        kernel.last_exec_time_ns = res.exec_time_ns
        kernel.last_res = res
    return full


kernel.last_exec_time_ns = None
kernel.last_res = None
